# revision 12
# baseline (speedup 1.0000x reference)
"""Attention2D Trainium2 Bass kernel.

Reference computation (per sample s of 4):
    x  = GroupNorm32(q[s])                      # [512, 4096] (c, hw)
    qp = Wq xn + bq ; kp = Wk xn + bk ; vp = Wv xn + bv
    S[i, j]  = sum_c kp[c, i] qp[c, j] / sqrt(512)
    A[:, j]  = softmax_i(S[:, j])
    out[c,j] = sum_i vp[c, i] A[i, j]
    y        = (Wo out + bo + q[s]) / sqrt(2)

Sharding: 8 cores = 4 samples x 2 query-halves (2048 tokens each).
The host permutes the token axis per core so the core's query half is
always tokens [0:2048) -> every core runs an identical program (SPMD,
no collectives).  Key/value work over all 4096 tokens is duplicated
between the two cores of a sample (cheap relative to attention).

On-chip layout: scores are computed as S[i(keys on partitions), j] so
that exp() is a single ScalarE pass PSUM->SBUF and the softmax
denominator Z[j] = sum_i E[i, j] is a ones-vector matmul on TensorE --
no transposes anywhere.  V is produced directly transposed (vfT[i, c])
by swapping matmul operands.  All matmul operands are bf16 (full PE
speed); accumulation in fp32 PSUM; GroupNorm stats in fp32.
"""

import numpy as np
import ml_dtypes

import concourse.bass as bass
import concourse.bacc as bacc
import concourse.tile as tile
import concourse.mybir as mybir
from concourse.bass_utils import run_bass_kernel_spmd

F32 = mybir.dt.float32
BF16 = mybir.dt.bfloat16
AF = mybir.ActivationFunctionType

P = 128          # partitions
C = 512          # channels
CT = C // P      # channel tiles (4)
T = 4096         # tokens per sample (h*w)
NQ = 2048        # query tokens per core
JC = 512         # query chunk (PSUM bank width in fp32)
NJ = NQ // JC    # query chunks per core (4)
IT = T // P      # key tiles (32)
NG_TILE = 8      # groups per channel tile (32 groups / 4 tiles)
GS = 16          # channels per group
EPS = 1e-6
SCALE = 1.0 / np.sqrt(C)
INV_SQRT2 = 0.7071067811865476


def _emit_body(nc, pools, aps, nj=NJ, do_attn=True):
    """One full forward pass. `pools` are long-lived tile pools; PSUM usage
    never exceeds 8 banks (pss 2 + psav 4 + psz 2)."""
    (const, big, sbx, st, sbe, sbw, sbq, sby, pss, psav, psz) = pools
    (x_d, xb_d, y_d, w_sb, vec_sb, bvrep, indf, indb, ones_bf, ones_f1,
     ones_fc, eps_t) = aps

    # ---- persistent activations ----
    xn = big.tile([P, CT, T], BF16, tag="xn")     # normalized input
    kf = big.tile([P, CT, T], BF16, tag="kf")     # K  [c, i]
    qf = big.tile([P, CT, NQ], BF16, tag="qf")    # Q  [c, j]
    vfT = big.tile([P, IT, C], BF16, tag="vfT")   # V^T [i, c]

    # ================= phase 1: GroupNorm =================
    # stats + normalization read the host-cast bf16 copy of x (half the HBM
    # traffic of f32; stats arithmetic stays f32)
    x_ts = []
    for t in range(CT):
        x_t = sbx.tile([P, T], BF16, tag="x", name=f"x{t}")
        # DMA in halves so stats can start on the first half early
        nc.sync.dma_start(
            out=x_t[:, 0:T // 2], in_=xb_d[t * P:(t + 1) * P, 0:T // 2])
        nc.sync.dma_start(
            out=x_t[:, T // 2:T], in_=xb_d[t * P:(t + 1) * P, T // 2:T])
        x_ts.append(x_t)
    for t in range(CT):
        x_t = x_ts[t]
        # raw per-channel sums: sum(x) on DVE, sum(x^2) on ACT (parallel
        # engines, per half so each overlaps the other half's DMA)
        parts = st.tile([P, 4], F32, tag="parts")
        for h in range(2):
            sl = slice(h * (T // 2), (h + 1) * (T // 2))
            nc.vector.reduce_sum(
                out=parts[:, h:h + 1], in_=x_t[:, sl],
                axis=mybir.AxisListType.X)
            # Square's full output is scratch; aim it at the xn slice that
            # the later (same-engine) normalize overwrites -> no extra SBUF
            nc.scalar.activation(
                out=xn[:, t, sl], in_=x_t[:, sl], func=AF.Square,
                accum_out=parts[:, 2 + h:3 + h])
        # t2 = [sum(x), sum(x^2)] ; indf carries the 1/(16*4096) factor
        t2 = st.tile([P, 2], F32, tag="t2")
        nc.vector.tensor_add(t2[:, 0:1], parts[:, 0:1], parts[:, 1:2])
        nc.vector.tensor_add(t2[:, 1:2], parts[:, 2:3], parts[:, 3:4])
        # group-reduce -> [mean_g, E[x^2]_g]
        g_ps = psz.tile([NG_TILE, 2], F32, tag="z")
        nc.tensor.matmul(g_ps, indf, t2, start=True, stop=True)
        gm = st.tile([NG_TILE, 2], F32, tag="gm")
        nc.vector.tensor_copy(out=gm, in_=g_ps)
        var = st.tile([NG_TILE, 1], F32, tag="var")
        nc.vector.tensor_mul(var, gm[:, 0:1], gm[:, 0:1])
        nc.vector.tensor_sub(var, gm[:, 1:2], var)
        sd = st.tile([NG_TILE, 1], F32, tag="sd")
        nc.scalar.activation(
            out=sd, in_=var, func=AF.Sqrt, bias=eps_t[0:NG_TILE, :]
        )
        rstd = st.tile([NG_TILE, 1], F32, tag="rstd")
        nc.vector.reciprocal(out=rstd, in_=sd)
        gm2 = st.tile([NG_TILE, 2], F32, tag="gm2")
        nc.vector.tensor_copy(out=gm2[:, 0:1], in_=gm[:, 0:1])
        nc.vector.tensor_copy(out=gm2[:, 1:2], in_=rstd)
        # broadcast group stats back to channels
        bc_ps = psz.tile([P, 2], F32, tag="z")
        nc.tensor.matmul(bc_ps, indb, gm2, start=True, stop=True)
        a_t = st.tile([P, 1], F32, tag="a")
        b_t = st.tile([P, 1], F32, tag="b")
        nc.vector.tensor_mul(a_t, bc_ps[:, 1:2], vec_sb["gamma"][:, t:t + 1])
        nc.vector.tensor_mul(b_t, bc_ps[:, 0:1], a_t)
        nc.vector.tensor_sub(b_t, vec_sb["beta"][:, t:t + 1], b_t)
        # normalize in halves (finer pipelining into the QKV phase)
        for h in range(2):
            nc.scalar.activation(
                out=xn[:, t, h * (T // 2):(h + 1) * (T // 2)],
                in_=x_t[:, h * (T // 2):(h + 1) * (T // 2)],
                func=AF.Identity, bias=b_t, scale=a_t,
            )

    # ================= phase 2: Q/K/V projections =================
    # PSUM: alternate between the two pools -> 6 effective buffers.
    def proj_psum(idx, shape):
        pool, tag = ((pss, "s"), (psav, "av"))[idx % 2]
        return pool.tile(shape, F32, tag=tag, name=f"pp{idx % 2}")

    # K: kf[c_out, i] ; Q: qf[c_out, j] (queries are tokens 0:NQ)
    pidx = 0
    for t_out in range(CT):
        for ic in range(T // JC):
            kp = proj_psum(pidx, [P, JC]); pidx += 1
            for t in range(CT):
                nc.tensor.matmul(
                    kp,
                    w_sb["wkT"][:, t, t_out * P:(t_out + 1) * P],
                    xn[:, t, ic * JC:(ic + 1) * JC],
                    start=(t == 0), stop=(t == CT - 1),
                )
            nc.scalar.activation(
                out=kf[:, t_out, ic * JC:(ic + 1) * JC], in_=kp,
                func=AF.Identity, bias=vec_sb["bk"][:, t_out:t_out + 1],
            )
    for t_out in range(CT):
        for jc in range(NJ):
            qp = proj_psum(pidx, [P, JC]); pidx += 1
            for t in range(CT):
                nc.tensor.matmul(
                    qp,
                    w_sb["wqT"][:, t, t_out * P:(t_out + 1) * P],
                    xn[:, t, jc * JC:(jc + 1) * JC],
                    start=(t == 0), stop=(t == CT - 1),
                )
            nc.scalar.activation(
                out=qf[:, t_out, jc * JC:(jc + 1) * JC], in_=qp,
                func=AF.Identity, bias=vec_sb["bq"][:, t_out:t_out + 1],
            )
    # V, produced transposed: vfT[i, c] = sum_c' xn[c', i] wvT[c', c]
    for k in range(IT):
        vp = proj_psum(pidx, [P, C]); pidx += 1
        for t in range(CT):
            nc.tensor.matmul(
                vp,
                xn[:, t, k * P:(k + 1) * P],
                w_sb["wvT"][:, t, :],
                start=(t == 0), stop=(t == CT - 1),
            )
        nc.vector.tensor_add(vfT[:, k, :], vp, bvrep)

    # ================= phase 3: attention =================
    if not do_attn:
        return
    for jc in range(nj):
        # residual input for this chunk (original x, queries 0:NQ)
        xqs = []
        for mo in range(CT):
            xq_t = sbq.tile([P, JC], F32, tag="xq")
            nc.sync.dma_start(
                out=xq_t,
                in_=x_d[mo * P:(mo + 1) * P, jc * JC:(jc + 1) * JC],
            )
            xs = sbq.tile([P, JC], F32, tag="xqs")
            # on DVE so phase-3 ScalarE stays pure-Exp (no LUT reloads)
            nc.vector.tensor_scalar(
                out=xs, in0=xq_t,
                scalar1=INV_SQRT2, scalar2=vec_sb["bo2"][:, mo:mo + 1],
                op0=mybir.AluOpType.mult, op1=mybir.AluOpType.add,
            )
            xqs.append(xs)

        zacc = sbw.tile([P, JC], F32, tag="zacc")
        av_ps = [psav.tile([P, JC], F32, tag="av", name=f"av{m}") for m in range(CT)]

        def av_step(k, e_t):
            if k == 0:
                nc.vector.tensor_copy(out=zacc, in_=e_t)
            else:
                nc.vector.tensor_add(zacc, zacc, e_t)
            for m in range(CT):
                nc.tensor.matmul(
                    av_ps[m],
                    vfT[:, k, m * P:(m + 1) * P],
                    e_t,
                    start=(k == 0), stop=(k == IT - 1),
                )

        prev_e = None
        for k in range(IT):
            s_ps = pss.tile([P, JC], F32, tag="s")
            for t in range(CT):
                nc.tensor.matmul(
                    s_ps,
                    kf[:, t, k * P:(k + 1) * P],
                    qf[:, t, jc * JC:(jc + 1) * JC],
                    start=(t == 0), stop=(t == CT - 1),
                )
            e_t = sbe.tile([P, JC], BF16, tag="e")
            nc.scalar.activation(out=e_t, in_=s_ps, func=AF.Exp, scale=SCALE)
            if prev_e is not None:
                av_step(k - 1, prev_e)
            prev_e = e_t
        av_step(IT - 1, prev_e)
        # normalize: out_n = av / Z  (Z = cross-partition sum of zacc via PE)
        z_ps = psz.tile([1, JC], F32, tag="z")
        nc.tensor.matmul(z_ps, ones_fc, zacc, start=True, stop=True)
        zinv = sbw.tile([1, JC], F32, tag="zinv")
        nc.vector.reciprocal(out=zinv, in_=z_ps)
        zbc_ps = psz.tile([P, JC], F32, tag="z")
        nc.tensor.matmul(zbc_ps, ones_f1, zinv, start=True, stop=True)
        zrep = sbw.tile([P, JC], F32, tag="zrep")
        nc.vector.tensor_copy(out=zrep, in_=zbc_ps)
        out_n = sbw.tile([P, CT, JC], BF16, tag="outn")
        for m in range(CT):
            nc.vector.tensor_mul(out_n[:, m, :], av_ps[m], zrep)
        # output projection + residual (PSUM from psz so next chunk's AV
        # accumulators don't contend with this chunk's tail)
        for mo in range(CT):
            y_ps = psz.tile([P, JC], F32, tag="z")
            for m in range(CT):
                nc.tensor.matmul(
                    y_ps,
                    w_sb["woT"][:, m, mo * P:(mo + 1) * P],
                    out_n[:, m, :],
                    start=(m == 0), stop=(m == CT - 1),
                )
            yf = sby.tile([P, JC], F32, tag="y")
            nc.vector.scalar_tensor_tensor(
                out=yf, in0=y_ps, scalar=INV_SQRT2, in1=xqs[mo],
                op0=mybir.AluOpType.mult, op1=mybir.AluOpType.add,
            )
            nc.sync.dma_start(
                out=y_d[mo * P:(mo + 1) * P, jc * JC:(jc + 1) * JC],
                in_=yf,
            )


def build_bass(loop_iters=None, nj=NJ, do_attn=True):
    """loop_iters=None: single-shot kernel.  loop_iters=R: wrap the body in a
    hardware For_i loop (for on-device timing; everything re-runs each
    iteration, output is idempotent)."""
    nc = bacc.Bacc("TRN2", target_bir_lowering=False, debug=False)

    x_d = nc.dram_tensor("x", [C, T], F32, kind="ExternalInput").ap()
    xb_d = nc.dram_tensor("xb", [C, T], BF16, kind="ExternalInput").ap()
    w_d = {
        n: nc.dram_tensor(n, [C, C], BF16, kind="ExternalInput").ap()
        for n in ("wqT", "wkT", "wvT", "woT")
    }
    # per-channel vectors in [p, tile] layout (c = t*128 + p)
    vec_d = {
        n: nc.dram_tensor(n, [P, CT], F32, kind="ExternalInput").ap()
        for n in ("bq", "bk", "bo2", "gamma", "beta")
    }
    bvrep_d = nc.dram_tensor("bv_rep", [P, C], F32, kind="ExternalInput").ap()
    indf_d = nc.dram_tensor("indf", [P, NG_TILE], F32, kind="ExternalInput").ap()
    indb_d = nc.dram_tensor("indb", [NG_TILE, P], F32, kind="ExternalInput").ap()
    onesb_d = nc.dram_tensor("ones_bf", [P, 1], BF16, kind="ExternalInput").ap()
    onesc_d = nc.dram_tensor("ones_fc", [P, 1], F32, kind="ExternalInput").ap()
    onesf_d = nc.dram_tensor("ones_f1", [1, P], F32, kind="ExternalInput").ap()
    y_d = nc.dram_tensor("y", [C, NQ], F32, kind="ExternalOutput").ap()

    with tile.TileContext(nc) as tc:
        with (
            tc.tile_pool(name="const", bufs=1) as const,
            tc.tile_pool(name="big", bufs=1) as big,
            tc.tile_pool(name="sbx", bufs=4) as sbx,
            tc.tile_pool(name="st", bufs=2) as st,
            tc.tile_pool(name="sbe", bufs=3) as sbe,
            tc.tile_pool(name="sbw", bufs=2) as sbw,
            tc.tile_pool(name="sbq", bufs=3) as sbq,
            tc.tile_pool(name="sby", bufs=4) as sby,
            tc.tile_pool(name="pss", bufs=2, space="PSUM") as pss,
            tc.tile_pool(name="psav", bufs=4, space="PSUM") as psav,
            tc.tile_pool(name="psz", bufs=2, space="PSUM") as psz,
        ):
            # ---- constants (loaded once, outside any timing loop) ----
            w_sb = {}
            for n, d in w_d.items():
                w_sb[n] = const.tile([P, CT, C], BF16, tag=n, name=n)
                nc.gpsimd.dma_start(out=w_sb[n], in_=d.rearrange("(t p) o -> p t o", p=P))
            vec_sb = {}
            for n, d in vec_d.items():
                vec_sb[n] = const.tile([P, CT], F32, tag=n, name=n)
                nc.gpsimd.dma_start(out=vec_sb[n], in_=d)
            bvrep = const.tile([P, C], F32, tag="bvrep")
            nc.gpsimd.dma_start(out=bvrep, in_=bvrep_d)
            indf = const.tile([P, NG_TILE], F32, tag="indf")
            nc.gpsimd.dma_start(out=indf, in_=indf_d)
            indb = const.tile([NG_TILE, P], F32, tag="indb")
            nc.gpsimd.dma_start(out=indb, in_=indb_d)
            ones_bf = const.tile([P, 1], BF16, tag="onesb")
            nc.gpsimd.dma_start(out=ones_bf, in_=onesb_d)
            ones_f1 = const.tile([1, P], F32, tag="onesf")
            nc.gpsimd.dma_start(out=ones_f1, in_=onesf_d)
            ones_fc = const.tile([P, 1], F32, tag="onesc")
            nc.gpsimd.dma_start(out=ones_fc, in_=onesc_d)
            eps_t = const.tile([P, 1], F32, tag="eps")
            nc.vector.memset(eps_t, EPS)

            pools = (const, big, sbx, st, sbe, sbw, sbq, sby, pss, psav, psz)
            aps = (x_d, xb_d, y_d, w_sb, vec_sb, bvrep, indf, indb,
                   ones_bf, ones_f1, ones_fc, eps_t)
            if loop_iters is None:
                _emit_body(nc, pools, aps, nj=nj, do_attn=do_attn)
            else:
                with tc.For_i(0, loop_iters, 1):
                    _emit_body(nc, pools, aps, nj=nj, do_attn=do_attn)
    nc.compile()
    return nc


def make_in_maps(q, gamma, beta, wq, bq, wk, bk, wv, bv, wo, bo):
    """Host-side prep: per-core permuted x + replicated (pre-transposed) weights."""
    f32 = np.float32
    bf16 = ml_dtypes.bfloat16
    q = np.asarray(q, f32)
    b = q.shape[0]
    x = q.reshape(b, C, T)

    def pt(v):  # [512] -> [128, 4] (c = t*128 + p)
        return np.ascontiguousarray(np.asarray(v, f32).reshape(CT, P).T)

    common = {
        "wqT": np.ascontiguousarray(np.asarray(wq, f32).T).astype(bf16),
        "wkT": np.ascontiguousarray(np.asarray(wk, f32).T).astype(bf16),
        "wvT": np.ascontiguousarray(np.asarray(wv, f32).T).astype(bf16),
        "woT": np.ascontiguousarray(np.asarray(wo, f32).T).astype(bf16),
        "bq": pt(bq), "bk": pt(bk), "bo2": pt(np.asarray(bo, f32) * INV_SQRT2),
        "gamma": pt(gamma), "beta": pt(beta),
        "bv_rep": np.ascontiguousarray(
            np.broadcast_to(np.asarray(bv, f32), (P, C))),
        "indf": np.ascontiguousarray(
            (np.arange(P)[:, None] // GS == np.arange(NG_TILE)[None, :])
            .astype(f32) / (GS * T)),
        "indb": np.ascontiguousarray(
            (np.arange(P)[None, :] // GS == np.arange(NG_TILE)[:, None])
            .astype(f32)),
        "ones_bf": np.ones((P, 1), bf16),
        "ones_fc": np.ones((P, 1), f32),
        "ones_f1": np.ones((1, P), f32),
    }
    in_maps = []
    for core in range(8):
        s, half = divmod(core, 2)
        xs = x[s]
        if half == 0:
            xp = xs
        else:
            xp = np.concatenate([xs[:, NQ:], xs[:, :NQ]], axis=1)
        xpc = np.ascontiguousarray(xp)
        in_maps.append({"x": xpc, "xb": xpc.astype(bf16), **common})
    return in_maps


def assemble_output(results, b=4, h=64, w=64):
    out = np.empty((b, C, T), np.float32)
    for core in range(8):
        s, half = divmod(core, 2)
        out[s][:, half * NQ:(half + 1) * NQ] = results[core]["y"]
    return out.reshape(b, C, h, w)


_NC = None


def get_nc():
    global _NC
    if _NC is None:
        _NC = build_bass()
    return _NC


def kernel(**inputs):
    in_maps = make_in_maps(**inputs)
    nc = get_nc()
    res = run_bass_kernel_spmd(nc, in_maps, core_ids=list(range(8)))
    return assemble_output(res.results)


if __name__ == "__main__":
    nc = get_nc()
    print("built + compiled ok")


# revision 14
# speedup vs baseline: 1.4908x; 1.4908x over previous
"""Attention2D Trainium2 Bass kernel.

Reference computation (per sample s of 4):
    x  = GroupNorm32(q[s])                      # [512, 4096] (c, hw)
    qp = Wq xn + bq ; kp = Wk xn + bk ; vp = Wv xn + bv
    S[i, j]  = sum_c kp[c, i] qp[c, j] / sqrt(512)
    A[:, j]  = softmax_i(S[:, j])
    out[c,j] = sum_i vp[c, i] A[i, j]
    y        = (Wo out + bo + q[s]) / sqrt(2)

Sharding: 8 cores = 4 samples x 2 query-halves (2048 tokens each).
The host permutes the token axis per core so the core's query half is
always tokens [0:2048) -> every core runs an identical program (SPMD,
no collectives).  Key/value work over all 4096 tokens is duplicated
between the two cores of a sample (cheap relative to attention).

On-chip layout: scores are computed as S[i(keys on partitions), j] so
that exp() is a single ScalarE pass PSUM->SBUF and the softmax
denominator Z[j] = sum_i E[i, j] is a ones-vector matmul on TensorE --
no transposes anywhere.  V is produced directly transposed (vfT[i, c])
by swapping matmul operands.  All matmul operands are bf16 (full PE
speed); accumulation in fp32 PSUM; GroupNorm stats in fp32.
"""

import numpy as np
import ml_dtypes

import concourse.bass as bass
import concourse.bacc as bacc
import concourse.tile as tile
import concourse.mybir as mybir
from concourse.bass_utils import run_bass_kernel_spmd

F32 = mybir.dt.float32
BF16 = mybir.dt.bfloat16
AF = mybir.ActivationFunctionType

P = 128          # partitions
C = 512          # channels
CT = C // P      # channel tiles (4)
T = 4096         # tokens per sample (h*w)
NQ = 2048        # query tokens per core
JC = 512         # query chunk (PSUM bank width in fp32)
NJ = NQ // JC    # query chunks per core (4)
IT = T // P      # key tiles (32)
NG_TILE = 8      # groups per channel tile (32 groups / 4 tiles)
GS = 16          # channels per group
EPS = 1e-6
SCALE = 1.0 / np.sqrt(C)
INV_SQRT2 = 0.7071067811865476
GN_MODE = "sums"  # "sums" (reduce+Square) or "bn" (bn_stats)


def _emit_body(nc, pools, aps, nj=NJ, do_attn=True):
    """One full forward pass. `pools` are long-lived tile pools; PSUM usage
    never exceeds 8 banks (pss 2 + psav 4 + psz 2)."""
    (const, big, sbx, st, sbe, sbw, sbq, sby, pss, psav, psz) = pools
    (x_d, xb_d, y_d, w_sb, vec_sb, bvrep, indf, indb, ones_bf, ones_f1,
     ones_fc, eps_t) = aps

    # ---- persistent activations ----
    xn = big.tile([P, CT, T], BF16, tag="xn")     # normalized input
    kf = big.tile([P, CT, T], BF16, tag="kf")     # K  [c, i]
    qf = big.tile([P, CT, NQ], BF16, tag="qf")    # Q  [c, j]
    vfT = big.tile([P, IT, C], BF16, tag="vfT")   # V^T [i, c]

    # ================= phase 1: GroupNorm =================
    # stats + normalization read the host-cast bf16 copy of x (half the HBM
    # traffic of f32; stats arithmetic stays f32)
    x_ts = []
    for t in range(CT):
        x_t = sbx.tile([P, T], BF16, tag="x", name=f"x{t}")
        # DMA in halves so stats can start on the first half early
        nc.sync.dma_start(
            out=x_t[:, 0:T // 2], in_=xb_d[t * P:(t + 1) * P, 0:T // 2])
        nc.sync.dma_start(
            out=x_t[:, T // 2:T], in_=xb_d[t * P:(t + 1) * P, T // 2:T])
        x_ts.append(x_t)
    for t in range(CT):
        x_t = x_ts[t]
        t2 = st.tile([P, 2], F32, tag="t2")
        if GN_MODE == "sums":
            # raw per-channel sums: sum(x) on DVE, sum(x^2) on ACT (parallel
            # engines, per half so each overlaps the other half's DMA)
            parts = st.tile([P, 4], F32, tag="parts")
            for h in range(2):
                sl = slice(h * (T // 2), (h + 1) * (T // 2))
                nc.vector.reduce_sum(
                    out=parts[:, h:h + 1], in_=x_t[:, sl],
                    axis=mybir.AxisListType.X)
                # Square's full output is scratch; aim it at the xn slice
                # that the later (same-engine) normalize overwrites
                nc.scalar.activation(
                    out=xn[:, t, sl], in_=x_t[:, sl], func=AF.Square,
                    accum_out=parts[:, 2 + h:3 + h])
            # t2 = [sum(x), sum(x^2)]; indf carries the 1/(16*4096) factor
            nc.vector.tensor_add(t2[:, 0:1], parts[:, 0:1], parts[:, 1:2])
            nc.vector.tensor_add(t2[:, 1:2], parts[:, 2:3], parts[:, 3:4])
        else:
            stats = st.tile([P, 8, 6], F32, tag="stats")
            for sg in range(8):
                nc.vector.bn_stats(
                    out=stats[:, sg, :], in_=x_t[:, sg * 512:(sg + 1) * 512])
            mv = st.tile([P, 2], F32, tag="mv")
            nc.vector.bn_aggr(out=mv, in_=stats)
            # t2 = [mean, E[x^2]] * (GS*T) to match the indf scaling
            nc.vector.tensor_scalar_mul(
                out=t2[:, 0:1], in0=mv[:, 0:1], scalar1=float(GS * T))
            nc.vector.tensor_mul(t2[:, 1:2], mv[:, 0:1], mv[:, 0:1])
            nc.vector.tensor_add(t2[:, 1:2], t2[:, 1:2], mv[:, 1:2])
            nc.vector.tensor_scalar_mul(
                out=t2[:, 1:2], in0=t2[:, 1:2], scalar1=float(GS * T))
        # group-reduce -> [mean_g, E[x^2]_g]
        g_ps = psz.tile([NG_TILE, 2], F32, tag="z")
        nc.tensor.matmul(g_ps, indf, t2, start=True, stop=True)
        gm = st.tile([NG_TILE, 2], F32, tag="gm")
        nc.vector.tensor_copy(out=gm, in_=g_ps)
        var = st.tile([NG_TILE, 1], F32, tag="var")
        nc.vector.tensor_mul(var, gm[:, 0:1], gm[:, 0:1])
        nc.vector.tensor_sub(var, gm[:, 1:2], var)
        sd = st.tile([NG_TILE, 1], F32, tag="sd")
        nc.scalar.activation(
            out=sd, in_=var, func=AF.Sqrt, bias=eps_t[0:NG_TILE, :]
        )
        rstd = st.tile([NG_TILE, 1], F32, tag="rstd")
        nc.vector.reciprocal(out=rstd, in_=sd)
        gm2 = st.tile([NG_TILE, 2], F32, tag="gm2")
        nc.vector.tensor_copy(out=gm2[:, 0:1], in_=gm[:, 0:1])
        nc.vector.tensor_copy(out=gm2[:, 1:2], in_=rstd)
        # broadcast group stats back to channels
        bc_ps = psz.tile([P, 2], F32, tag="z")
        nc.tensor.matmul(bc_ps, indb, gm2, start=True, stop=True)
        a_t = st.tile([P, 1], F32, tag="a")
        b_t = st.tile([P, 1], F32, tag="b")
        nc.vector.tensor_mul(a_t, bc_ps[:, 1:2], vec_sb["gamma"][:, t:t + 1])
        nc.vector.tensor_mul(b_t, bc_ps[:, 0:1], a_t)
        nc.vector.tensor_sub(b_t, vec_sb["beta"][:, t:t + 1], b_t)
        # normalize in halves (finer pipelining into the QKV phase)
        for h in range(2):
            nc.scalar.activation(
                out=xn[:, t, h * (T // 2):(h + 1) * (T // 2)],
                in_=x_t[:, h * (T // 2):(h + 1) * (T // 2)],
                func=AF.Identity, bias=b_t, scale=a_t,
            )

    # ================= phase 2: Q/K/V projections =================
    # PSUM: alternate between the two pools -> 6 effective buffers.
    def proj_psum(idx, shape):
        pool, tag = ((pss, "s"), (psav, "av"))[idx % 2]
        return pool.tile(shape, F32, tag=tag, name=f"pp{idx % 2}")

    # K: kf[c_out, i] ; Q: qf[c_out, j] (queries are tokens 0:NQ)
    pidx = 0
    for t_out in range(CT):
        for ic in range(T // JC):
            kp = proj_psum(pidx, [P, JC]); pidx += 1
            for t in range(CT):
                nc.tensor.matmul(
                    kp,
                    w_sb["wkT"][:, t, t_out * P:(t_out + 1) * P],
                    xn[:, t, ic * JC:(ic + 1) * JC],
                    start=(t == 0), stop=(t == CT - 1),
                )
            nc.scalar.activation(
                out=kf[:, t_out, ic * JC:(ic + 1) * JC], in_=kp,
                func=AF.Identity, bias=vec_sb["bk"][:, t_out:t_out + 1],
            )
    for t_out in range(CT):
        for jc in range(NJ):
            qp = proj_psum(pidx, [P, JC]); pidx += 1
            for t in range(CT):
                nc.tensor.matmul(
                    qp,
                    w_sb["wqT"][:, t, t_out * P:(t_out + 1) * P],
                    xn[:, t, jc * JC:(jc + 1) * JC],
                    start=(t == 0), stop=(t == CT - 1),
                )
            nc.scalar.activation(
                out=qf[:, t_out, jc * JC:(jc + 1) * JC], in_=qp,
                func=AF.Identity, bias=vec_sb["bq"][:, t_out:t_out + 1],
            )
    # V, produced transposed: vfT[i, c] = sum_c' xn[c', i] wvT[c', c]
    for k in range(IT):
        vp = proj_psum(pidx, [P, C]); pidx += 1
        for t in range(CT):
            nc.tensor.matmul(
                vp,
                xn[:, t, k * P:(k + 1) * P],
                w_sb["wvT"][:, t, :],
                start=(t == 0), stop=(t == CT - 1),
            )
        nc.vector.tensor_add(vfT[:, k, :], vp, bvrep)

    # ================= phase 3: attention =================
    if not do_attn:
        return
    for jc in range(nj):
        # residual input for this chunk (original x, queries 0:NQ)
        xqs = []
        for mo in range(CT):
            xq_t = sbq.tile([P, JC], F32, tag="xq")
            nc.sync.dma_start(
                out=xq_t,
                in_=x_d[mo * P:(mo + 1) * P, jc * JC:(jc + 1) * JC],
            )
            xs = sbq.tile([P, JC], F32, tag="xqs")
            # on DVE so phase-3 ScalarE stays pure-Exp (no LUT reloads)
            nc.vector.tensor_scalar(
                out=xs, in0=xq_t,
                scalar1=INV_SQRT2, scalar2=vec_sb["bo2"][:, mo:mo + 1],
                op0=mybir.AluOpType.mult, op1=mybir.AluOpType.add,
            )
            xqs.append(xs)

        zacc = sbw.tile([P, JC], F32, tag="zacc")
        av_ps = [psav.tile([P, JC], F32, tag="av", name=f"av{m}") for m in range(CT)]

        def av_step(k, e_t):
            if k == 0:
                nc.vector.tensor_copy(out=zacc, in_=e_t)
            else:
                nc.vector.tensor_add(zacc, zacc, e_t)
            for m in range(CT):
                nc.tensor.matmul(
                    av_ps[m],
                    vfT[:, k, m * P:(m + 1) * P],
                    e_t,
                    start=(k == 0), stop=(k == IT - 1),
                )

        prev_e = None
        for k in range(IT):
            s_ps = pss.tile([P, JC], F32, tag="s")
            for t in range(CT):
                nc.tensor.matmul(
                    s_ps,
                    kf[:, t, k * P:(k + 1) * P],
                    qf[:, t, jc * JC:(jc + 1) * JC],
                    start=(t == 0), stop=(t == CT - 1),
                )
            e_t = sbe.tile([P, JC], BF16, tag="e")
            nc.scalar.activation(out=e_t, in_=s_ps, func=AF.Exp, scale=SCALE)
            if prev_e is not None:
                av_step(k - 1, prev_e)
            prev_e = e_t
        av_step(IT - 1, prev_e)
        # normalize: out_n = av / Z  (Z = cross-partition sum of zacc via PE)
        z_ps = psz.tile([1, JC], F32, tag="z")
        nc.tensor.matmul(z_ps, ones_fc, zacc, start=True, stop=True)
        zinv = sbw.tile([1, JC], F32, tag="zinv")
        nc.vector.reciprocal(out=zinv, in_=z_ps)
        zbc_ps = psz.tile([P, JC], F32, tag="z")
        nc.tensor.matmul(zbc_ps, ones_f1, zinv, start=True, stop=True)
        zrep = sbw.tile([P, JC], F32, tag="zrep")
        nc.vector.tensor_copy(out=zrep, in_=zbc_ps)
        out_n = sbw.tile([P, CT, JC], BF16, tag="outn")
        for m in range(CT):
            nc.vector.tensor_mul(out_n[:, m, :], av_ps[m], zrep)
        # output projection + residual (PSUM from psz so next chunk's AV
        # accumulators don't contend with this chunk's tail)
        for mo in range(CT):
            y_ps = psz.tile([P, JC], F32, tag="z")
            for m in range(CT):
                nc.tensor.matmul(
                    y_ps,
                    w_sb["woT"][:, m, mo * P:(mo + 1) * P],
                    out_n[:, m, :],
                    start=(m == 0), stop=(m == CT - 1),
                )
            yf = sby.tile([P, JC], F32, tag="y")
            nc.vector.scalar_tensor_tensor(
                out=yf, in0=y_ps, scalar=INV_SQRT2, in1=xqs[mo],
                op0=mybir.AluOpType.mult, op1=mybir.AluOpType.add,
            )
            nc.sync.dma_start(
                out=y_d[mo * P:(mo + 1) * P, jc * JC:(jc + 1) * JC],
                in_=yf,
            )


def build_bass(loop_iters=None, nj=NJ, do_attn=True, gn_mode=None):
    global GN_MODE
    if gn_mode is not None:
        GN_MODE = gn_mode
    """loop_iters=None: single-shot kernel.  loop_iters=R: wrap the body in a
    hardware For_i loop (for on-device timing; everything re-runs each
    iteration, output is idempotent)."""
    nc = bacc.Bacc("TRN2", target_bir_lowering=False, debug=False)

    x_d = nc.dram_tensor("x", [C, T], F32, kind="ExternalInput").ap()
    xb_d = nc.dram_tensor("xb", [C, T], BF16, kind="ExternalInput").ap()
    w_d = {
        n: nc.dram_tensor(n, [C, C], BF16, kind="ExternalInput").ap()
        for n in ("wqT", "wkT", "wvT", "woT")
    }
    # per-channel vectors in [p, tile] layout (c = t*128 + p)
    vec_d = {
        n: nc.dram_tensor(n, [P, CT], F32, kind="ExternalInput").ap()
        for n in ("bq", "bk", "bo2", "gamma", "beta")
    }
    bvrep_d = nc.dram_tensor("bv_rep", [P, C], F32, kind="ExternalInput").ap()
    indf_d = nc.dram_tensor("indf", [P, NG_TILE], F32, kind="ExternalInput").ap()
    indb_d = nc.dram_tensor("indb", [NG_TILE, P], F32, kind="ExternalInput").ap()
    onesb_d = nc.dram_tensor("ones_bf", [P, 1], BF16, kind="ExternalInput").ap()
    onesc_d = nc.dram_tensor("ones_fc", [P, 1], F32, kind="ExternalInput").ap()
    onesf_d = nc.dram_tensor("ones_f1", [1, P], F32, kind="ExternalInput").ap()
    y_d = nc.dram_tensor("y", [C, NQ], F32, kind="ExternalOutput").ap()

    with tile.TileContext(nc) as tc:
        with (
            tc.tile_pool(name="const", bufs=1) as const,
            tc.tile_pool(name="big", bufs=1) as big,
            tc.tile_pool(name="sbx", bufs=4) as sbx,
            tc.tile_pool(name="st", bufs=2) as st,
            tc.tile_pool(name="sbe", bufs=3) as sbe,
            tc.tile_pool(name="sbw", bufs=2) as sbw,
            tc.tile_pool(name="sbq", bufs=3) as sbq,
            tc.tile_pool(name="sby", bufs=4) as sby,
            tc.tile_pool(name="pss", bufs=2, space="PSUM") as pss,
            tc.tile_pool(name="psav", bufs=4, space="PSUM") as psav,
            tc.tile_pool(name="psz", bufs=2, space="PSUM") as psz,
        ):
            # ---- constants (loaded once, outside any timing loop) ----
            w_sb = {}
            for n, d in w_d.items():
                w_sb[n] = const.tile([P, CT, C], BF16, tag=n, name=n)
                nc.gpsimd.dma_start(out=w_sb[n], in_=d.rearrange("(t p) o -> p t o", p=P))
            vec_sb = {}
            for n, d in vec_d.items():
                vec_sb[n] = const.tile([P, CT], F32, tag=n, name=n)
                nc.gpsimd.dma_start(out=vec_sb[n], in_=d)
            bvrep = const.tile([P, C], F32, tag="bvrep")
            nc.gpsimd.dma_start(out=bvrep, in_=bvrep_d)
            indf = const.tile([P, NG_TILE], F32, tag="indf")
            nc.gpsimd.dma_start(out=indf, in_=indf_d)
            indb = const.tile([NG_TILE, P], F32, tag="indb")
            nc.gpsimd.dma_start(out=indb, in_=indb_d)
            ones_bf = const.tile([P, 1], BF16, tag="onesb")
            nc.gpsimd.dma_start(out=ones_bf, in_=onesb_d)
            ones_f1 = const.tile([1, P], F32, tag="onesf")
            nc.gpsimd.dma_start(out=ones_f1, in_=onesf_d)
            ones_fc = const.tile([P, 1], F32, tag="onesc")
            nc.gpsimd.dma_start(out=ones_fc, in_=onesc_d)
            eps_t = const.tile([P, 1], F32, tag="eps")
            nc.vector.memset(eps_t, EPS)

            pools = (const, big, sbx, st, sbe, sbw, sbq, sby, pss, psav, psz)
            aps = (x_d, xb_d, y_d, w_sb, vec_sb, bvrep, indf, indb,
                   ones_bf, ones_f1, ones_fc, eps_t)
            if loop_iters is None:
                _emit_body(nc, pools, aps, nj=nj, do_attn=do_attn)
            else:
                with tc.For_i(0, loop_iters, 1):
                    _emit_body(nc, pools, aps, nj=nj, do_attn=do_attn)
    nc.compile()
    return nc


def make_in_maps(q, gamma, beta, wq, bq, wk, bk, wv, bv, wo, bo):
    """Host-side prep: per-core permuted x + replicated (pre-transposed) weights."""
    f32 = np.float32
    bf16 = ml_dtypes.bfloat16
    q = np.asarray(q, f32)
    b = q.shape[0]
    x = q.reshape(b, C, T)

    def pt(v):  # [512] -> [128, 4] (c = t*128 + p)
        return np.ascontiguousarray(np.asarray(v, f32).reshape(CT, P).T)

    common = {
        "wqT": np.ascontiguousarray(np.asarray(wq, f32).T).astype(bf16),
        "wkT": np.ascontiguousarray(np.asarray(wk, f32).T).astype(bf16),
        "wvT": np.ascontiguousarray(np.asarray(wv, f32).T).astype(bf16),
        "woT": np.ascontiguousarray(np.asarray(wo, f32).T).astype(bf16),
        "bq": pt(bq), "bk": pt(bk), "bo2": pt(np.asarray(bo, f32) * INV_SQRT2),
        "gamma": pt(gamma), "beta": pt(beta),
        "bv_rep": np.ascontiguousarray(
            np.broadcast_to(np.asarray(bv, f32), (P, C))),
        "indf": np.ascontiguousarray(
            (np.arange(P)[:, None] // GS == np.arange(NG_TILE)[None, :])
            .astype(f32) / (GS * T)),
        "indb": np.ascontiguousarray(
            (np.arange(P)[None, :] // GS == np.arange(NG_TILE)[:, None])
            .astype(f32)),
        "ones_bf": np.ones((P, 1), bf16),
        "ones_fc": np.ones((P, 1), f32),
        "ones_f1": np.ones((1, P), f32),
    }
    in_maps = []
    for core in range(8):
        s, half = divmod(core, 2)
        xs = x[s]
        if half == 0:
            xp = xs
        else:
            xp = np.concatenate([xs[:, NQ:], xs[:, :NQ]], axis=1)
        xpc = np.ascontiguousarray(xp)
        in_maps.append({"x": xpc, "xb": xpc.astype(bf16), **common})
    return in_maps


def assemble_output(results, b=4, h=64, w=64):
    out = np.empty((b, C, T), np.float32)
    for core in range(8):
        s, half = divmod(core, 2)
        out[s][:, half * NQ:(half + 1) * NQ] = results[core]["y"]
    return out.reshape(b, C, h, w)


_NC = None


def get_nc():
    global _NC
    if _NC is None:
        _NC = build_bass()
    return _NC


def kernel(**inputs):
    in_maps = make_in_maps(**inputs)
    nc = get_nc()
    try:
        res = run_bass_kernel_spmd(nc, in_maps, core_ids=list(range(8)))
    except Exception:
        # transient NRT device wedges have been observed; one retry usually
        # succeeds after the runtime resets the core
        res = run_bass_kernel_spmd(nc, in_maps, core_ids=list(range(8)))
    return assemble_output(res.results)


if __name__ == "__main__":
    nc = get_nc()
    print("built + compiled ok")


# revision 19
# speedup vs baseline: 1.5693x; 1.0527x over previous
"""Attention2D Trainium2 Bass kernel.

Reference computation (per sample s of 4):
    x  = GroupNorm32(q[s])                      # [512, 4096] (c, hw)
    qp = Wq xn + bq ; kp = Wk xn + bk ; vp = Wv xn + bv
    S[i, j]  = sum_c kp[c, i] qp[c, j] / sqrt(512)
    A[:, j]  = softmax_i(S[:, j])
    out[c,j] = sum_i vp[c, i] A[i, j]
    y        = (Wo out + bo + q[s]) / sqrt(2)

Sharding: 8 cores = 4 samples x 2 query-halves (2048 tokens each).
The host permutes the token axis per core so the core's query half is
always tokens [0:2048) -> every core runs an identical program (SPMD,
no collectives).  Key/value work over all 4096 tokens is duplicated
between the two cores of a sample (cheap relative to attention).

On-chip layout: scores are computed as S[i(keys on partitions), j] so
that exp() is a single ScalarE pass PSUM->SBUF and the softmax
denominator Z[j] = sum_i E[i, j] is a ones-vector matmul on TensorE --
no transposes anywhere.  V is produced directly transposed (vfT[i, c])
by swapping matmul operands.  All matmul operands are bf16 (full PE
speed); accumulation in fp32 PSUM; GroupNorm stats in fp32.
"""

import numpy as np
import ml_dtypes

import concourse.bass as bass
import concourse.bacc as bacc
import concourse.tile as tile
import concourse.mybir as mybir
from concourse.bass_utils import run_bass_kernel_spmd

F32 = mybir.dt.float32
BF16 = mybir.dt.bfloat16
AF = mybir.ActivationFunctionType

P = 128          # partitions
C = 512          # channels
CT = C // P      # channel tiles (4)
T = 4096         # tokens per sample (h*w)
NQ = 2048        # query tokens per core
JC = 512         # query chunk (PSUM bank width in fp32)
NJ = NQ // JC    # query chunks per core (4)
IT = T // P      # key tiles (32)
NG_TILE = 8      # groups per channel tile (32 groups / 4 tiles)
GS = 16          # channels per group
EPS = 1e-6
SCALE = 1.0 / np.sqrt(C)
INV_SQRT2 = 0.7071067811865476
GN_MODE = "sums"  # "sums" (reduce+Square) or "bn" (bn_stats)


def _emit_body(nc, pools, aps, nj=NJ, do_attn=True):
    """One full forward pass. `pools` are long-lived tile pools; PSUM usage
    never exceeds 8 banks (pss 2 + psav 4 + psz 2)."""
    (const, big, sbx, st, sbe, sbw, sbq, sby, pss, psav, psz) = pools
    (x_d, xb_d, y_d, w_sb, vec_sb, bv_row, indf, indb, ones_f1,
     ones_fc, eps_t) = aps

    # ---- persistent activations ----
    kf = big.tile([P, CT, T], BF16, tag="kf")     # K  [c, i]
    qf = big.tile([P, CT, NQ], BF16, tag="qf")    # Q  [c, j]
    vfT = big.tile([P, IT, C], BF16, tag="vfT")   # V^T [i, c]
    # GroupNorm is folded into the projections: w2 = w * diag(a) per c_in
    w2 = {n: big.tile([P, CT, C], BF16, tag=f"w2{n}", name=f"w2{n}")
          for n in ("wqT", "wkT", "wvT")}

    # ================= phase 1: GroupNorm =================
    # stats + normalization read the host-cast bf16 copy of x (half the HBM
    # traffic of f32; stats arithmetic stays f32)
    x_ts = []
    for t in range(CT):
        x_t = sbx.tile([P, T], BF16, tag="x", name=f"x{t}")
        # DMA in halves so stats can start on the first half early
        nc.sync.dma_start(
            out=x_t[:, 0:T // 2], in_=xb_d[t * P:(t + 1) * P, 0:T // 2])
        nc.sync.dma_start(
            out=x_t[:, T // 2:T], in_=xb_d[t * P:(t + 1) * P, T // 2:T])
        x_ts.append(x_t)
    b_bfs = []
    gm_all = st.tile([NG_TILE, 2, CT], F32, tag="gm_all")   # [Mg | rstd] x tile
    var_all = st.tile([NG_TILE, CT], F32, tag="var_all")
    for t in range(CT):
        x_t = x_ts[t]
        t2 = st.tile([P, 2], F32, tag="t2")
        if GN_MODE == "sums":
            # raw per-channel sums: sum(x) on DVE, sum(x^2) on ACT (parallel
            # engines, per half so each overlaps the other half's DMA)
            parts = st.tile([P, 4], F32, tag="parts")
            for h in range(2):
                sl = slice(h * (T // 2), (h + 1) * (T // 2))
                nc.vector.reduce_sum(
                    out=parts[:, h:h + 1], in_=x_t[:, sl],
                    axis=mybir.AxisListType.X)
                sq_scr = st.tile([P, T // 2], BF16, tag="sqscr")
                nc.scalar.activation(
                    out=sq_scr, in_=x_t[:, sl], func=AF.Square,
                    accum_out=parts[:, 2 + h:3 + h])
            # t2 = [sum(x), sum(x^2)]; indf carries the 1/(16*4096) factor
            nc.vector.tensor_add(t2[:, 0:1], parts[:, 0:1], parts[:, 1:2])
            nc.vector.tensor_add(t2[:, 1:2], parts[:, 2:3], parts[:, 3:4])
        else:
            stats = st.tile([P, 8, 6], F32, tag="stats")
            for sg in range(8):
                nc.vector.bn_stats(
                    out=stats[:, sg, :], in_=x_t[:, sg * 512:(sg + 1) * 512])
            mv = st.tile([P, 2], F32, tag="mv")
            nc.vector.bn_aggr(out=mv, in_=stats)
            # t2 = [mean, E[x^2]] * (GS*T) to match the indf scaling
            nc.vector.tensor_scalar_mul(
                out=t2[:, 0:1], in0=mv[:, 0:1], scalar1=float(GS * T))
            nc.vector.tensor_mul(t2[:, 1:2], mv[:, 0:1], mv[:, 0:1])
            nc.vector.tensor_add(t2[:, 1:2], t2[:, 1:2], mv[:, 1:2])
            nc.vector.tensor_scalar_mul(
                out=t2[:, 1:2], in0=t2[:, 1:2], scalar1=float(GS * T))
        # group-reduce -> [mean_g, E[x^2]_g]
        g_ps = psz.tile([NG_TILE, 2], F32, tag="z")
        nc.tensor.matmul(g_ps, indf, t2, start=True, stop=True)
        nc.vector.tensor_copy(out=gm_all[:, :, t:t + 1], in_=g_ps.rearrange("p (s o) -> p s o", o=1))
        nc.vector.tensor_mul(
            var_all[:, t:t + 1], gm_all[:, 0, t:t + 1], gm_all[:, 0, t:t + 1])
        nc.vector.tensor_sub(
            var_all[:, t:t + 1], gm_all[:, 1, t:t + 1], var_all[:, t:t + 1])

    # batched tail: ONE Sqrt / reciprocal / broadcast for all 4 tiles
    # (avoids ScalarE LUT reloads between Square/Sqrt/Identity)
    sd_all = st.tile([NG_TILE, CT], F32, tag="sd_all")
    nc.scalar.activation(
        out=sd_all, in_=var_all, func=AF.Sqrt, bias=eps_t[0:NG_TILE, :])
    nc.vector.reciprocal(out=gm_all[:, 1, :], in_=sd_all)
    bc_ps = psz.tile([P, 2, CT], F32, tag="z")
    nc.tensor.matmul(bc_ps, indb, gm_all, start=True, stop=True)
    for t in range(CT):
        a_t = st.tile([P, 1], F32, tag="a", bufs=4)
        b_t = st.tile([P, 1], F32, tag="b", bufs=6)
        nc.vector.tensor_mul(
            a_t, bc_ps[:, 1, t:t + 1], vec_sb["gamma"][:, t:t + 1])
        nc.vector.tensor_mul(b_t, bc_ps[:, 0, t:t + 1], a_t)
        nc.vector.tensor_sub(b_t, vec_sb["beta"][:, t:t + 1], b_t)
        b_bf = st.tile([P, 1], BF16, tag="bbf", bufs=6)
        nc.vector.tensor_copy(out=b_bf, in_=b_t)
        b_bfs.append(b_bf)
        # fold the GN affine scale into the projection weights (per c_in row)
        for n in ("wqT", "wkT", "wvT"):
            nc.scalar.activation(
                out=w2[n][:, t, :], in_=w_sb[n][:, t, :],
                func=AF.Identity, scale=a_t)

    # ---- fold the GN shift into the projection biases: b' = b + W @ b_gn
    b2 = {}
    for n, bias_name in (("wqT", "bq"), ("wkT", "bk")):
        b2t = st.tile([P, CT], F32, tag=f"b2{n}", name=f"b2{n}")
        for mo in range(CT):
            bp = psz.tile([P, 1], F32, tag="z", name="bp")
            for t in range(CT):
                nc.tensor.matmul(
                    bp, w_sb[n][:, t, mo * P:(mo + 1) * P], b_bfs[t],
                    start=(t == 0), stop=(t == CT - 1))
            nc.vector.tensor_add(
                b2t[:, mo:mo + 1], bp, vec_sb[bias_name][:, mo:mo + 1])
        b2[n] = b2t
    # V bias as a row, broadcast across partitions via PE
    bvr_ps = psz.tile([1, C], F32, tag="z")
    for t in range(CT):
        nc.tensor.matmul(bvr_ps, b_bfs[t], w_sb["wvT"][:, t, :],
                         start=(t == 0), stop=(t == CT - 1))
    bvrow_sb = st.tile([1, C], F32, tag="bvrow")
    nc.vector.tensor_add(bvrow_sb, bvr_ps, bv_row)
    bvrep_ps = psz.tile([P, C], F32, tag="z")
    nc.tensor.matmul(bvrep_ps, ones_f1, bvrow_sb, start=True, stop=True)
    bvrep = sbw.tile([P, C], F32, tag="bvrep")
    nc.vector.tensor_copy(out=bvrep, in_=bvrep_ps)

    # ================= phase 2: Q/K/V projections =================
    # PSUM: alternate between the two pools -> 6 effective buffers.
    def proj_psum(idx, shape):
        pool, tag = ((pss, "s"), (psav, "av"))[idx % 2]
        return pool.tile(shape, F32, tag=tag, name=f"pp{idx % 2}")

    # K: kf[c_out, i] ; Q: qf[c_out, j] (queries are tokens 0:NQ)
    pidx = 0
    for t_out in range(CT):
        for ic in range(T // JC):
            kp = proj_psum(pidx, [P, JC]); pidx += 1
            for t in range(CT):
                nc.tensor.matmul(
                    kp,
                    w2["wkT"][:, t, t_out * P:(t_out + 1) * P],
                    x_ts[t][:, ic * JC:(ic + 1) * JC],
                    start=(t == 0), stop=(t == CT - 1),
                )
            nc.scalar.activation(
                out=kf[:, t_out, ic * JC:(ic + 1) * JC], in_=kp,
                func=AF.Identity, bias=b2["wkT"][:, t_out:t_out + 1],
            )
    for t_out in range(CT):
        for jc in range(NJ):
            qp = proj_psum(pidx, [P, JC]); pidx += 1
            for t in range(CT):
                nc.tensor.matmul(
                    qp,
                    w2["wqT"][:, t, t_out * P:(t_out + 1) * P],
                    x_ts[t][:, jc * JC:(jc + 1) * JC],
                    start=(t == 0), stop=(t == CT - 1),
                )
            nc.scalar.activation(
                out=qf[:, t_out, jc * JC:(jc + 1) * JC], in_=qp,
                func=AF.Identity, bias=b2["wqT"][:, t_out:t_out + 1],
            )
    # V, produced transposed: vfT[i, c] = sum_c' xn[c', i] wvT[c', c]
    for k in range(IT):
        vp = proj_psum(pidx, [P, C]); pidx += 1
        for t in range(CT):
            nc.tensor.matmul(
                vp,
                x_ts[t][:, k * P:(k + 1) * P],
                w2["wvT"][:, t, :],
                start=(t == 0), stop=(t == CT - 1),
            )
        nc.vector.tensor_add(vfT[:, k, :], vp, bvrep)

    # ================= phase 3: attention =================
    if not do_attn:
        return
    for jc in range(nj):
        # residual input for this chunk (original x, queries 0:NQ)
        xqs = []
        for mo in range(CT):
            xq_t = sbq.tile([P, JC], F32, tag="xq")
            nc.sync.dma_start(
                out=xq_t,
                in_=x_d[mo * P:(mo + 1) * P, jc * JC:(jc + 1) * JC],
            )
            xs = sbq.tile([P, JC], F32, tag="xqs")
            # on DVE so phase-3 ScalarE stays pure-Exp (no LUT reloads)
            nc.vector.tensor_scalar(
                out=xs, in0=xq_t,
                scalar1=INV_SQRT2, scalar2=vec_sb["bo2"][:, mo:mo + 1],
                op0=mybir.AluOpType.mult, op1=mybir.AluOpType.add,
            )
            xqs.append(xs)

        zacc = sbw.tile([P, JC], F32, tag="zacc")
        av_ps = [psav.tile([P, JC], F32, tag="av", name=f"av{m}") for m in range(CT)]

        def av_step(k, e_t):
            if k == 0:
                nc.vector.tensor_copy(out=zacc, in_=e_t)
            else:
                nc.vector.tensor_add(zacc, zacc, e_t)
            for m in range(CT):
                nc.tensor.matmul(
                    av_ps[m],
                    vfT[:, k, m * P:(m + 1) * P],
                    e_t,
                    start=(k == 0), stop=(k == IT - 1),
                )

        prev_e = None
        for k in range(IT):
            s_ps = pss.tile([P, JC], F32, tag="s")
            for t in range(CT):
                nc.tensor.matmul(
                    s_ps,
                    kf[:, t, k * P:(k + 1) * P],
                    qf[:, t, jc * JC:(jc + 1) * JC],
                    start=(t == 0), stop=(t == CT - 1),
                )
            e_t = sbe.tile([P, JC], BF16, tag="e")
            nc.scalar.activation(out=e_t, in_=s_ps, func=AF.Exp, scale=SCALE)
            if prev_e is not None:
                av_step(k - 1, prev_e)
            prev_e = e_t
        av_step(IT - 1, prev_e)
        # normalize: out_n = av / Z  (Z = cross-partition sum of zacc via
        # PE; broadcast BEFORE reciprocal so the two matmuls are
        # back-to-back and only one DVE hop remains)
        z_ps = psz.tile([1, JC], F32, tag="z")
        nc.tensor.matmul(z_ps, ones_fc, zacc, start=True, stop=True)
        zr_sb = sbw.tile([1, JC], F32, tag="zinv")
        nc.vector.tensor_copy(out=zr_sb, in_=z_ps)
        zbc_ps = psz.tile([P, JC], F32, tag="z")
        nc.tensor.matmul(zbc_ps, ones_f1, zr_sb, start=True, stop=True)
        zrep = sbw.tile([P, JC], F32, tag="zrep")
        nc.vector.reciprocal(out=zrep, in_=zbc_ps)
        out_n = sbw.tile([P, CT, JC], BF16, tag="outn")
        for m in range(CT):
            nc.vector.tensor_mul(out_n[:, m, :], av_ps[m], zrep)
        # output projection + residual (PSUM from psz so next chunk's AV
        # accumulators don't contend with this chunk's tail)
        for mo in range(CT):
            y_ps = psz.tile([P, JC], F32, tag="z")
            for m in range(CT):
                nc.tensor.matmul(
                    y_ps,
                    w_sb["woT"][:, m, mo * P:(mo + 1) * P],
                    out_n[:, m, :],
                    start=(m == 0), stop=(m == CT - 1),
                )
            yf = sby.tile([P, JC], F32, tag="y")
            nc.vector.scalar_tensor_tensor(
                out=yf, in0=y_ps, scalar=INV_SQRT2, in1=xqs[mo],
                op0=mybir.AluOpType.mult, op1=mybir.AluOpType.add,
            )
            nc.sync.dma_start(
                out=y_d[mo * P:(mo + 1) * P, jc * JC:(jc + 1) * JC],
                in_=yf,
            )


def build_bass(loop_iters=None, nj=NJ, do_attn=True, gn_mode=None):
    global GN_MODE
    if gn_mode is not None:
        GN_MODE = gn_mode
    """loop_iters=None: single-shot kernel.  loop_iters=R: wrap the body in a
    hardware For_i loop (for on-device timing; everything re-runs each
    iteration, output is idempotent)."""
    nc = bacc.Bacc("TRN2", target_bir_lowering=False, debug=False)

    x_d = nc.dram_tensor("x", [C, T], F32, kind="ExternalInput").ap()
    xb_d = nc.dram_tensor("xb", [C, T], BF16, kind="ExternalInput").ap()
    w_d = {
        n: nc.dram_tensor(n, [C, C], BF16, kind="ExternalInput").ap()
        for n in ("wqT", "wkT", "wvT", "woT")
    }
    # per-channel vectors in [p, tile] layout (c = t*128 + p)
    vec_d = {
        n: nc.dram_tensor(n, [P, CT], F32, kind="ExternalInput").ap()
        for n in ("bq", "bk", "bo2", "gamma", "beta")
    }
    bvrow_d = nc.dram_tensor("bv_row", [1, C], F32, kind="ExternalInput").ap()
    indf_d = nc.dram_tensor("indf", [P, NG_TILE], F32, kind="ExternalInput").ap()
    indb_d = nc.dram_tensor("indb", [NG_TILE, P], F32, kind="ExternalInput").ap()
    onesc_d = nc.dram_tensor("ones_fc", [P, 1], F32, kind="ExternalInput").ap()
    onesf_d = nc.dram_tensor("ones_f1", [1, P], F32, kind="ExternalInput").ap()
    y_d = nc.dram_tensor("y", [C, NQ], F32, kind="ExternalOutput").ap()

    with tile.TileContext(nc) as tc:
        with (
            tc.tile_pool(name="const", bufs=1) as const,
            tc.tile_pool(name="big", bufs=1) as big,
            tc.tile_pool(name="sbx", bufs=4) as sbx,
            tc.tile_pool(name="st", bufs=2) as st,
            tc.tile_pool(name="sbe", bufs=3) as sbe,
            tc.tile_pool(name="sbw", bufs=2) as sbw,
            tc.tile_pool(name="sbq", bufs=3) as sbq,
            tc.tile_pool(name="sby", bufs=4) as sby,
            tc.tile_pool(name="pss", bufs=2, space="PSUM") as pss,
            tc.tile_pool(name="psav", bufs=4, space="PSUM") as psav,
            tc.tile_pool(name="psz", bufs=2, space="PSUM") as psz,
        ):
            # ---- constants (loaded once, outside any timing loop; small
            # ones first -- the GN group matmuls need them early, while the
            # big weights aren't read until ~18us in) ----
            indf = const.tile([P, NG_TILE], F32, tag="indf")
            nc.gpsimd.dma_start(out=indf, in_=indf_d)
            indb = const.tile([NG_TILE, P], F32, tag="indb")
            nc.gpsimd.dma_start(out=indb, in_=indb_d)
            vec_sb = {}
            for n, d in vec_d.items():
                vec_sb[n] = const.tile([P, CT], F32, tag=n, name=n)
                nc.gpsimd.dma_start(out=vec_sb[n], in_=d)
            bv_row = const.tile([1, C], F32, tag="bvrow_c")
            nc.gpsimd.dma_start(out=bv_row, in_=bvrow_d)
            ones_f1 = const.tile([1, P], F32, tag="onesf")
            nc.gpsimd.dma_start(out=ones_f1, in_=onesf_d)
            ones_fc = const.tile([P, 1], F32, tag="onesc")
            nc.gpsimd.dma_start(out=ones_fc, in_=onesc_d)
            eps_t = const.tile([P, 1], F32, tag="eps")
            nc.vector.memset(eps_t, EPS)
            w_sb = {}
            for n, d in w_d.items():
                w_sb[n] = const.tile([P, CT, C], BF16, tag=n, name=n)
                nc.gpsimd.dma_start(out=w_sb[n], in_=d.rearrange("(t p) o -> p t o", p=P))

            pools = (const, big, sbx, st, sbe, sbw, sbq, sby, pss, psav, psz)
            aps = (x_d, xb_d, y_d, w_sb, vec_sb, bv_row, indf, indb,
                   ones_f1, ones_fc, eps_t)
            if loop_iters is None:
                _emit_body(nc, pools, aps, nj=nj, do_attn=do_attn)
            else:
                with tc.For_i(0, loop_iters, 1):
                    _emit_body(nc, pools, aps, nj=nj, do_attn=do_attn)
    nc.compile()
    return nc


def make_in_maps(q, gamma, beta, wq, bq, wk, bk, wv, bv, wo, bo):
    """Host-side prep: per-core permuted x + replicated (pre-transposed) weights."""
    f32 = np.float32
    bf16 = ml_dtypes.bfloat16
    q = np.asarray(q, f32)
    b = q.shape[0]
    x = q.reshape(b, C, T)

    def pt(v):  # [512] -> [128, 4] (c = t*128 + p)
        return np.ascontiguousarray(np.asarray(v, f32).reshape(CT, P).T)

    common = {
        "wqT": np.ascontiguousarray(np.asarray(wq, f32).T).astype(bf16),
        "wkT": np.ascontiguousarray(np.asarray(wk, f32).T).astype(bf16),
        "wvT": np.ascontiguousarray(np.asarray(wv, f32).T).astype(bf16),
        "woT": np.ascontiguousarray(np.asarray(wo, f32).T).astype(bf16),
        "bq": pt(bq), "bk": pt(bk), "bo2": pt(np.asarray(bo, f32) * INV_SQRT2),
        "gamma": pt(gamma), "beta": pt(beta),
        "bv_row": np.ascontiguousarray(np.asarray(bv, f32).reshape(1, C)),
        "indf": np.ascontiguousarray(
            (np.arange(P)[:, None] // GS == np.arange(NG_TILE)[None, :])
            .astype(f32) / (GS * T)),
        "indb": np.ascontiguousarray(
            (np.arange(P)[None, :] // GS == np.arange(NG_TILE)[:, None])
            .astype(f32)),
        "ones_fc": np.ones((P, 1), f32),
        "ones_f1": np.ones((1, P), f32),
    }
    in_maps = []
    for core in range(8):
        s, half = divmod(core, 2)
        xs = x[s]
        if half == 0:
            xp = xs
        else:
            xp = np.concatenate([xs[:, NQ:], xs[:, :NQ]], axis=1)
        xpc = np.ascontiguousarray(xp)
        in_maps.append({"x": xpc, "xb": xpc.astype(bf16), **common})
    return in_maps


def assemble_output(results, b=4, h=64, w=64):
    out = np.empty((b, C, T), np.float32)
    for core in range(8):
        s, half = divmod(core, 2)
        out[s][:, half * NQ:(half + 1) * NQ] = results[core]["y"]
    return out.reshape(b, C, h, w)


_NC = None


def get_nc():
    global _NC
    if _NC is None:
        _NC = build_bass()
    return _NC


def kernel(**inputs):
    in_maps = make_in_maps(**inputs)
    nc = get_nc()
    try:
        res = run_bass_kernel_spmd(nc, in_maps, core_ids=list(range(8)))
    except Exception:
        # transient NRT device wedges have been observed; one retry usually
        # succeeds after the runtime resets the core
        res = run_bass_kernel_spmd(nc, in_maps, core_ids=list(range(8)))
    return assemble_output(res.results)


if __name__ == "__main__":
    nc = get_nc()
    print("built + compiled ok")


# revision 25
# speedup vs baseline: 1.6120x; 1.0272x over previous
"""Attention2D Trainium2 Bass kernel.

Reference computation (per sample s of 4):
    x  = GroupNorm32(q[s])                      # [512, 4096] (c, hw)
    qp = Wq xn + bq ; kp = Wk xn + bk ; vp = Wv xn + bv
    S[i, j]  = sum_c kp[c, i] qp[c, j] / sqrt(512)
    A[:, j]  = softmax_i(S[:, j])
    out[c,j] = sum_i vp[c, i] A[i, j]
    y        = (Wo out + bo + q[s]) / sqrt(2)

Sharding: 8 cores = 4 samples x 2 query-halves (2048 tokens each).
The host permutes the token axis per core so the core's query half is
always tokens [0:2048) -> every core runs an identical program (SPMD,
no collectives).  Key/value work over all 4096 tokens is duplicated
between the two cores of a sample (cheap relative to attention).

On-chip layout: scores are computed as S[i(keys on partitions), j] so
that exp() is a single ScalarE pass PSUM->SBUF and the softmax
denominator Z[j] = sum_i E[i, j] is a ones-vector matmul on TensorE --
no transposes anywhere.  V is produced directly transposed (vfT[i, c])
by swapping matmul operands.  All matmul operands are bf16 (full PE
speed); accumulation in fp32 PSUM; GroupNorm stats in fp32.
"""

import numpy as np
import ml_dtypes

import concourse.bass as bass
import concourse.bacc as bacc
import concourse.tile as tile
import concourse.mybir as mybir
from concourse.bass_utils import run_bass_kernel_spmd

F32 = mybir.dt.float32
BF16 = mybir.dt.bfloat16
AF = mybir.ActivationFunctionType

P = 128          # partitions
C = 512          # channels
CT = C // P      # channel tiles (4)
T = 4096         # tokens per sample (h*w)
NQ = 2048        # query tokens per core
JC = 512         # query chunk (PSUM bank width in fp32)
NJ = NQ // JC    # query chunks per core (4)
IT = T // P      # key tiles (32)
NG_TILE = 8      # groups per channel tile (32 groups / 4 tiles)
GS = 16          # channels per group
EPS = 1e-6
SCALE = 1.0 / np.sqrt(C)
INV_SQRT2 = 0.7071067811865476
GN_MODE = "sums"  # "sums" (reduce+Square) or "bn" (bn_stats)


def _emit_body(nc, pools, aps, nj=NJ, do_attn=True):
    """One full forward pass. `pools` are long-lived tile pools; PSUM usage
    never exceeds 8 banks (pss 2 + psav 4 + psz 2)."""
    (const, big, sbx, st, sbe, sbw, sbq, sby, pss, psav, psz) = pools
    (x_d, xb_d, y_d, w_sb, vec_sb, bv_row, indf, indb, ones_f1,
     ones_fc, eps_t) = aps

    # ---- persistent activations ----
    qf = big.tile([P, CT, NQ], BF16, tag="qf")    # Q  [c, j]
    vfT = big.tile([P, IT, C], BF16, tag="vfT")   # V^T [i, c]
    # GroupNorm is folded into the projections: w2 = w * diag(a) per c_in.
    # K is never projected: S = xn^T (Wk^T Wq) xn, with M0 = Wk^T Wq from
    # the host; m2 = diag(a) applied to M0^T rows here, the output-side
    # diag(a) applied in the Q-copy ACT scale.
    w2 = {n: big.tile([P, CT, C], BF16, tag=f"w2{n}", name=f"w2{n}")
          for n in ("m0T", "wvT")}

    # ================= phase 1: GroupNorm =================
    # stats + normalization read the host-cast bf16 copy of x (half the HBM
    # traffic of f32; stats arithmetic stays f32)
    x_ts = []
    for t in range(CT):
        x_t = sbx.tile([P, T], BF16, tag="x", name=f"x{t}")
        # DMA in halves so stats can start on the first half early
        nc.sync.dma_start(
            out=x_t[:, 0:T // 2], in_=xb_d[t * P:(t + 1) * P, 0:T // 2])
        nc.sync.dma_start(
            out=x_t[:, T // 2:T], in_=xb_d[t * P:(t + 1) * P, T // 2:T])
        x_ts.append(x_t)
    b_bfs = []
    a_ts = []
    gm_all = st.tile([NG_TILE, 2, CT], F32, tag="gm_all")   # [Mg | rstd] x tile
    var_all = st.tile([NG_TILE, CT], F32, tag="var_all")
    for t in range(CT):
        x_t = x_ts[t]
        t2 = st.tile([P, 2], F32, tag="t2")
        if GN_MODE == "sums":
            # raw per-channel sums: sum(x) on DVE, sum(x^2) on ACT (parallel
            # engines, per half so each overlaps the other half's DMA)
            parts = st.tile([P, 4], F32, tag="parts")
            for h in range(2):
                sl = slice(h * (T // 2), (h + 1) * (T // 2))
                nc.vector.reduce_sum(
                    out=parts[:, h:h + 1], in_=x_t[:, sl],
                    axis=mybir.AxisListType.X)
                sq_scr = st.tile([P, T // 2], BF16, tag="sqscr")
                nc.scalar.activation(
                    out=sq_scr, in_=x_t[:, sl], func=AF.Square,
                    accum_out=parts[:, 2 + h:3 + h])
            # t2 = [sum(x), sum(x^2)]; indf carries the 1/(16*4096) factor
            nc.vector.tensor_add(t2[:, 0:1], parts[:, 0:1], parts[:, 1:2])
            nc.vector.tensor_add(t2[:, 1:2], parts[:, 2:3], parts[:, 3:4])
        else:
            stats = st.tile([P, 8, 6], F32, tag="stats")
            for sg in range(8):
                nc.vector.bn_stats(
                    out=stats[:, sg, :], in_=x_t[:, sg * 512:(sg + 1) * 512])
            mv = st.tile([P, 2], F32, tag="mv")
            nc.vector.bn_aggr(out=mv, in_=stats)
            # t2 = [mean, E[x^2]] * (GS*T) to match the indf scaling
            nc.vector.tensor_scalar_mul(
                out=t2[:, 0:1], in0=mv[:, 0:1], scalar1=float(GS * T))
            nc.vector.tensor_mul(t2[:, 1:2], mv[:, 0:1], mv[:, 0:1])
            nc.vector.tensor_add(t2[:, 1:2], t2[:, 1:2], mv[:, 1:2])
            nc.vector.tensor_scalar_mul(
                out=t2[:, 1:2], in0=t2[:, 1:2], scalar1=float(GS * T))
        # group-reduce -> [mean_g, E[x^2]_g]
        g_ps = psz.tile([NG_TILE, 2], F32, tag="z")
        nc.tensor.matmul(g_ps, indf, t2, start=True, stop=True)
        nc.vector.tensor_copy(out=gm_all[:, :, t:t + 1], in_=g_ps.rearrange("p (s o) -> p s o", o=1))
        nc.vector.tensor_mul(
            var_all[:, t:t + 1], gm_all[:, 0, t:t + 1], gm_all[:, 0, t:t + 1])
        nc.vector.tensor_sub(
            var_all[:, t:t + 1], gm_all[:, 1, t:t + 1], var_all[:, t:t + 1])

    # batched tail: ONE Sqrt / reciprocal / broadcast for all 4 tiles
    # (avoids ScalarE LUT reloads between Square/Sqrt/Identity)
    sd_all = st.tile([NG_TILE, CT], F32, tag="sd_all")
    nc.scalar.activation(
        out=sd_all, in_=var_all, func=AF.Sqrt, bias=eps_t[0:NG_TILE, :])
    nc.vector.reciprocal(out=gm_all[:, 1, :], in_=sd_all)
    bc_ps = psz.tile([P, 2, CT], F32, tag="z")
    nc.tensor.matmul(bc_ps, indb, gm_all, start=True, stop=True)
    for t in range(CT):
        a_t = st.tile([P, 1], F32, tag="a", bufs=4)
        b_t = st.tile([P, 1], F32, tag="b", bufs=6)
        nc.vector.tensor_mul(
            a_t, bc_ps[:, 1, t:t + 1], vec_sb["gamma"][:, t:t + 1])
        nc.vector.tensor_mul(b_t, bc_ps[:, 0, t:t + 1], a_t)
        nc.vector.tensor_sub(b_t, vec_sb["beta"][:, t:t + 1], b_t)
        b_bf = st.tile([P, 1], BF16, tag="bbf", bufs=6)
        nc.vector.tensor_copy(out=b_bf, in_=b_t)
        b_bfs.append(b_bf)
        a_ts.append(a_t)
        # fold the GN affine scale into the projection weights (per c_in row)
        for n in ("m0T", "wvT"):
            nc.scalar.activation(
                out=w2[n][:, t, :], in_=w_sb[n][:, t, :],
                func=AF.Identity, scale=a_t)

    # ---- beta_q = bq + Wq @ b_gn  (the only projection bias that survives:
    # per-query score shifts cancel in softmax; the per-key shift r is
    # handled below as an exp() bias)
    bq_bf = st.tile([P, CT], BF16, tag="bq_bf")
    for mo in range(CT):
        bp = psz.tile([P, 1], F32, tag="z", name="bp")
        for t in range(CT):
            nc.tensor.matmul(
                bp, w_sb["wqT"][:, t, mo * P:(mo + 1) * P], b_bfs[t],
                start=(t == 0), stop=(t == CT - 1))
        tmp = st.tile([P, 1], F32, tag="bqtmp")
        nc.vector.tensor_add(tmp, bp, vec_sb["bq"][:, mo:mo + 1])
        nc.vector.tensor_copy(out=bq_bf[:, mo:mo + 1], in_=tmp)
    # v_r = a * (Wk^T beta_q)   (per-key shift direction)
    v_bf = st.tile([P, CT], BF16, tag="v_bf")
    for mt in range(CT):
        w1 = psz.tile([P, 1], F32, tag="z", name="w1")
        for kt in range(CT):
            nc.tensor.matmul(
                w1, w_sb["wkR"][:, kt, mt * P:(mt + 1) * P],
                bq_bf[:, kt:kt + 1],
                start=(kt == 0), stop=(kt == CT - 1))
        tmp2 = st.tile([P, 1], F32, tag="vtmp")
        nc.vector.tensor_mul(tmp2, w1, a_ts[mt])
        nc.vector.tensor_copy(out=v_bf[:, mt:mt + 1], in_=tmp2)
    # r[i] = v_r^T x_i for all keys, assembled as [128, IT] for exp bias
    # (row -> partition-major via a bf16 DMA transpose through DRAM)
    r_dram = nc.dram_tensor("r_scratch", [T], BF16, kind="ExternalOutput").ap()
    r_full = st.tile([1, T], BF16, tag="rfull")
    for ic in range(T // JC):
        r_ps = psz.tile([1, JC], F32, tag="z", name="rps")
        for t in range(CT):
            nc.tensor.matmul(
                r_ps, v_bf[:, t:t + 1], x_ts[t][:, ic * JC:(ic + 1) * JC],
                start=(t == 0), stop=(t == CT - 1))
        nc.vector.tensor_scalar_mul(
            out=r_full[:, ic * JC:(ic + 1) * JC], in0=r_ps, scalar1=SCALE)
    nc.sync.dma_start(
        out=r_dram.rearrange("(o t) -> o t", o=1), in_=r_full)
    r_ptb = st.tile([P, IT], BF16, tag="rptb")
    nc.sync.dma_start_transpose(
        out=r_ptb, in_=r_dram.rearrange("(k p) -> k p", p=P))
    r_pt = st.tile([P, IT], F32, tag="rpt")
    nc.vector.tensor_copy(out=r_pt, in_=r_ptb)
    # V bias as a row, broadcast across partitions via PE
    bvr_ps = psz.tile([1, C], F32, tag="z")
    for t in range(CT):
        nc.tensor.matmul(bvr_ps, b_bfs[t], w_sb["wvT"][:, t, :],
                         start=(t == 0), stop=(t == CT - 1))
    bvrow_sb = st.tile([1, C], F32, tag="bvrow")
    nc.vector.tensor_add(bvrow_sb, bvr_ps, bv_row)
    bvrep_ps = psz.tile([P, C], F32, tag="z")
    nc.tensor.matmul(bvrep_ps, ones_f1, bvrow_sb, start=True, stop=True)
    bvrep = sbw.tile([P, C], F32, tag="bvrep")
    nc.vector.tensor_copy(out=bvrep, in_=bvrep_ps)

    # ================= phase 2: Q/K/V projections =================
    # PSUM: alternate between the two pools -> 6 effective buffers.
    def proj_psum(idx, shape):
        pool, tag = ((pss, "s"), (psav, "av"))[idx % 2]
        return pool.tile(shape, F32, tag=tag, name=f"pp{idx % 2}")

    # Q'' = diag(a) M0 diag(a) x  (keys are raw x; no K projection)
    pidx = 0
    for t_out in range(CT):
        for jc in range(NJ):
            qp = proj_psum(pidx, [P, JC]); pidx += 1
            for t in range(CT):
                nc.tensor.matmul(
                    qp,
                    w2["m0T"][:, t, t_out * P:(t_out + 1) * P],
                    x_ts[t][:, jc * JC:(jc + 1) * JC],
                    start=(t == 0), stop=(t == CT - 1),
                )
            nc.scalar.activation(
                out=qf[:, t_out, jc * JC:(jc + 1) * JC], in_=qp,
                func=AF.Identity, scale=a_ts[t_out],
            )
    # V, produced transposed: vfT[i, c] = sum_c' xn[c', i] wvT[c', c]
    for k in range(IT):
        vp = proj_psum(pidx, [P, C]); pidx += 1
        for t in range(CT):
            nc.tensor.matmul(
                vp,
                x_ts[t][:, k * P:(k + 1) * P],
                w2["wvT"][:, t, :],
                start=(t == 0), stop=(t == CT - 1),
            )
        nc.vector.tensor_add(vfT[:, k, :], vp, bvrep)

    # ================= phase 3: attention =================
    if not do_attn:
        return
    for jc in range(nj):
        # residual input for this chunk (original x, queries 0:NQ)
        xqs = []
        for mo in range(CT):
            xq_t = sbq.tile([P, JC], F32, tag="xq")
            nc.sync.dma_start(
                out=xq_t,
                in_=x_d[mo * P:(mo + 1) * P, jc * JC:(jc + 1) * JC],
            )
            xs = sbq.tile([P, JC], F32, tag="xqs")
            # on DVE so phase-3 ScalarE stays pure-Exp (no LUT reloads)
            nc.vector.tensor_scalar(
                out=xs, in0=xq_t,
                scalar1=INV_SQRT2, scalar2=vec_sb["bo2"][:, mo:mo + 1],
                op0=mybir.AluOpType.mult, op1=mybir.AluOpType.add,
            )
            xqs.append(xs)

        zacc = sbw.tile([P, JC], F32, tag="zacc")
        av_ps = [psav.tile([P, JC], F32, tag="av", name=f"av{m}") for m in range(CT)]

        def av_step(k, e_t):
            if k == 0:
                nc.vector.tensor_copy(out=zacc, in_=e_t)
            else:
                nc.vector.tensor_add(zacc, zacc, e_t)
            for m in range(CT):
                nc.tensor.matmul(
                    av_ps[m],
                    vfT[:, k, m * P:(m + 1) * P],
                    e_t,
                    start=(k == 0), stop=(k == IT - 1),
                )

        prev_e = None
        for k in range(IT):
            s_ps = pss.tile([P, JC], F32, tag="s")
            for t in range(CT):
                nc.tensor.matmul(
                    s_ps,
                    x_ts[t][:, k * P:(k + 1) * P],
                    qf[:, t, jc * JC:(jc + 1) * JC],
                    start=(t == 0), stop=(t == CT - 1),
                )
            e_t = sbe.tile([P, JC], BF16, tag="e")
            nc.scalar.activation(out=e_t, in_=s_ps, func=AF.Exp, scale=SCALE,
                                 bias=r_pt[:, k:k + 1])
            if prev_e is not None:
                av_step(k - 1, prev_e)
            prev_e = e_t
        av_step(IT - 1, prev_e)
        # normalize: out_n = av / Z  (Z = cross-partition sum of zacc via
        # PE; broadcast BEFORE reciprocal so the two matmuls are
        # back-to-back and only one DVE hop remains)
        z_ps = psz.tile([1, JC], F32, tag="z")
        nc.tensor.matmul(z_ps, ones_fc, zacc, start=True, stop=True)
        zr_sb = sbw.tile([1, JC], F32, tag="zinv")
        nc.vector.tensor_copy(out=zr_sb, in_=z_ps)
        zbc_ps = psz.tile([P, JC], F32, tag="z")
        nc.tensor.matmul(zbc_ps, ones_f1, zr_sb, start=True, stop=True)
        zrep = sbw.tile([P, JC], F32, tag="zrep")
        nc.vector.reciprocal(out=zrep, in_=zbc_ps)
        out_n = sbw.tile([P, CT, JC], BF16, tag="outn")
        for m in range(CT):
            nc.vector.tensor_mul(out_n[:, m, :], av_ps[m], zrep)
        # output projection + residual (PSUM from psz so next chunk's AV
        # accumulators don't contend with this chunk's tail)
        for mo in range(CT):
            y_ps = psz.tile([P, JC], F32, tag="z")
            for m in range(CT):
                nc.tensor.matmul(
                    y_ps,
                    w_sb["woT"][:, m, mo * P:(mo + 1) * P],
                    out_n[:, m, :],
                    start=(m == 0), stop=(m == CT - 1),
                )
            yf = sby.tile([P, JC], F32, tag="y")
            nc.vector.scalar_tensor_tensor(
                out=yf, in0=y_ps, scalar=INV_SQRT2, in1=xqs[mo],
                op0=mybir.AluOpType.mult, op1=mybir.AluOpType.add,
            )
            nc.sync.dma_start(
                out=y_d[mo * P:(mo + 1) * P, jc * JC:(jc + 1) * JC],
                in_=yf,
            )


def build_bass(loop_iters=None, nj=NJ, do_attn=True, gn_mode=None):
    global GN_MODE
    if gn_mode is not None:
        GN_MODE = gn_mode
    """loop_iters=None: single-shot kernel.  loop_iters=R: wrap the body in a
    hardware For_i loop (for on-device timing; everything re-runs each
    iteration, output is idempotent)."""
    nc = bacc.Bacc("TRN2", target_bir_lowering=False, debug=False)

    x_d = nc.dram_tensor("x", [C, T], F32, kind="ExternalInput").ap()
    xb_d = nc.dram_tensor("xb", [C, T], BF16, kind="ExternalInput").ap()
    w_d = {
        n: nc.dram_tensor(n, [C, C], BF16, kind="ExternalInput").ap()
        for n in ("wqT", "wkR", "m0T", "wvT", "woT")
    }
    # per-channel vectors in [p, tile] layout (c = t*128 + p)
    vec_d = {
        n: nc.dram_tensor(n, [P, CT], F32, kind="ExternalInput").ap()
        for n in ("bq", "bk", "bo2", "gamma", "beta")
    }
    bvrow_d = nc.dram_tensor("bv_row", [1, C], F32, kind="ExternalInput").ap()
    indf_d = nc.dram_tensor("indf", [P, NG_TILE], F32, kind="ExternalInput").ap()
    indb_d = nc.dram_tensor("indb", [NG_TILE, P], F32, kind="ExternalInput").ap()
    onesc_d = nc.dram_tensor("ones_fc", [P, 1], F32, kind="ExternalInput").ap()
    onesf_d = nc.dram_tensor("ones_f1", [1, P], F32, kind="ExternalInput").ap()
    y_d = nc.dram_tensor("y", [C, NQ], F32, kind="ExternalOutput").ap()

    with tile.TileContext(nc) as tc:
        with (
            tc.tile_pool(name="const", bufs=1) as const,
            tc.tile_pool(name="big", bufs=1) as big,
            tc.tile_pool(name="sbx", bufs=4) as sbx,
            tc.tile_pool(name="st", bufs=2) as st,
            tc.tile_pool(name="sbe", bufs=3) as sbe,
            tc.tile_pool(name="sbw", bufs=2) as sbw,
            tc.tile_pool(name="sbq", bufs=3) as sbq,
            tc.tile_pool(name="sby", bufs=4) as sby,
            tc.tile_pool(name="pss", bufs=2, space="PSUM") as pss,
            tc.tile_pool(name="psav", bufs=4, space="PSUM") as psav,
            tc.tile_pool(name="psz", bufs=2, space="PSUM") as psz,
        ):
            # ---- constants (loaded once, outside any timing loop; small
            # ones first -- the GN group matmuls need them early, while the
            # big weights aren't read until ~18us in) ----
            indf = const.tile([P, NG_TILE], F32, tag="indf")
            nc.gpsimd.dma_start(out=indf, in_=indf_d)
            indb = const.tile([NG_TILE, P], F32, tag="indb")
            nc.gpsimd.dma_start(out=indb, in_=indb_d)
            vec_sb = {}
            for n, d in vec_d.items():
                vec_sb[n] = const.tile([P, CT], F32, tag=n, name=n)
                nc.gpsimd.dma_start(out=vec_sb[n], in_=d)
            bv_row = const.tile([1, C], F32, tag="bvrow_c")
            nc.gpsimd.dma_start(out=bv_row, in_=bvrow_d)
            ones_f1 = const.tile([1, P], F32, tag="onesf")
            nc.gpsimd.dma_start(out=ones_f1, in_=onesf_d)
            ones_fc = const.tile([P, 1], F32, tag="onesc")
            nc.gpsimd.dma_start(out=ones_fc, in_=onesc_d)
            eps_t = const.tile([P, 1], F32, tag="eps")
            nc.vector.memset(eps_t, EPS)
            w_sb = {}
            for n, d in w_d.items():
                w_sb[n] = const.tile([P, CT, C], BF16, tag=n, name=n)
                nc.gpsimd.dma_start(out=w_sb[n], in_=d.rearrange("(t p) o -> p t o", p=P))

            pools = (const, big, sbx, st, sbe, sbw, sbq, sby, pss, psav, psz)
            aps = (x_d, xb_d, y_d, w_sb, vec_sb, bv_row, indf, indb,
                   ones_f1, ones_fc, eps_t)
            if loop_iters is None:
                _emit_body(nc, pools, aps, nj=nj, do_attn=do_attn)
            else:
                with tc.For_i(0, loop_iters, 1):
                    _emit_body(nc, pools, aps, nj=nj, do_attn=do_attn)
    nc.compile()
    return nc


def make_in_maps(q, gamma, beta, wq, bq, wk, bk, wv, bv, wo, bo):
    """Host-side prep: per-core permuted x + replicated (pre-transposed) weights."""
    f32 = np.float32
    bf16 = ml_dtypes.bfloat16
    q = np.asarray(q, f32)
    b = q.shape[0]
    x = q.reshape(b, C, T)

    def pt(v):  # [512] -> [128, 4] (c = t*128 + p)
        return np.ascontiguousarray(np.asarray(v, f32).reshape(CT, P).T)

    common = {
        "wqT": np.ascontiguousarray(np.asarray(wq, f32).T).astype(bf16),
        "wkR": np.ascontiguousarray(np.asarray(wk, f32)).astype(bf16),
        "m0T": np.ascontiguousarray(
            (np.asarray(wq, f32).T @ np.asarray(wk, f32))).astype(bf16),
        "wvT": np.ascontiguousarray(np.asarray(wv, f32).T).astype(bf16),
        "woT": np.ascontiguousarray(np.asarray(wo, f32).T).astype(bf16),
        "bq": pt(bq), "bk": pt(bk), "bo2": pt(np.asarray(bo, f32) * INV_SQRT2),
        "gamma": pt(gamma), "beta": pt(beta),
        "bv_row": np.ascontiguousarray(np.asarray(bv, f32).reshape(1, C)),
        "indf": np.ascontiguousarray(
            (np.arange(P)[:, None] // GS == np.arange(NG_TILE)[None, :])
            .astype(f32) / (GS * T)),
        "indb": np.ascontiguousarray(
            (np.arange(P)[None, :] // GS == np.arange(NG_TILE)[:, None])
            .astype(f32)),
        "ones_fc": np.ones((P, 1), f32),
        "ones_f1": np.ones((1, P), f32),
    }
    in_maps = []
    for core in range(8):
        s, half = divmod(core, 2)
        xs = x[s]
        if half == 0:
            xp = xs
        else:
            xp = np.concatenate([xs[:, NQ:], xs[:, :NQ]], axis=1)
        xpc = np.ascontiguousarray(xp)
        in_maps.append({"x": xpc, "xb": xpc.astype(bf16), **common})
    return in_maps


def assemble_output(results, b=4, h=64, w=64):
    out = np.empty((b, C, T), np.float32)
    for core in range(8):
        s, half = divmod(core, 2)
        out[s][:, half * NQ:(half + 1) * NQ] = results[core]["y"]
    return out.reshape(b, C, h, w)


_NC = None


def get_nc():
    global _NC
    if _NC is None:
        _NC = build_bass()
    return _NC


def kernel(**inputs):
    in_maps = make_in_maps(**inputs)
    nc = get_nc()
    try:
        res = run_bass_kernel_spmd(nc, in_maps, core_ids=list(range(8)))
    except Exception:
        # transient NRT device wedges have been observed; one retry usually
        # succeeds after the runtime resets the core
        res = run_bass_kernel_spmd(nc, in_maps, core_ids=list(range(8)))
    return assemble_output(res.results)


if __name__ == "__main__":
    nc = get_nc()
    print("built + compiled ok")


# revision 28
# speedup vs baseline: 1.6584x; 1.0288x over previous
"""Attention2D Trainium2 Bass kernel.

Reference computation (per sample s of 4):
    x  = GroupNorm32(q[s])                      # [512, 4096] (c, hw)
    qp = Wq xn + bq ; kp = Wk xn + bk ; vp = Wv xn + bv
    S[i, j]  = sum_c kp[c, i] qp[c, j] / sqrt(512)
    A[:, j]  = softmax_i(S[:, j])
    out[c,j] = sum_i vp[c, i] A[i, j]
    y        = (Wo out + bo + q[s]) / sqrt(2)

Sharding: 8 cores = 4 samples x 2 query-halves (2048 tokens each).
The host permutes the token axis per core so the core's query half is
always tokens [0:2048) -> every core runs an identical program (SPMD,
no collectives).  Key/value work over all 4096 tokens is duplicated
between the two cores of a sample (cheap relative to attention).

On-chip layout: scores are computed as S[i(keys on partitions), j] so
that exp() is a single ScalarE pass PSUM->SBUF and the softmax
denominator Z[j] = sum_i E[i, j] is a ones-vector matmul on TensorE --
no transposes anywhere.  V is produced directly transposed (vfT[i, c])
by swapping matmul operands.  All matmul operands are bf16 (full PE
speed); accumulation in fp32 PSUM; GroupNorm stats in fp32.
"""

import numpy as np
import ml_dtypes

import concourse.bass as bass
import concourse.bacc as bacc
import concourse.tile as tile
import concourse.mybir as mybir
from concourse.bass_utils import run_bass_kernel_spmd

F32 = mybir.dt.float32
BF16 = mybir.dt.bfloat16
AF = mybir.ActivationFunctionType

P = 128          # partitions
C = 512          # channels
CT = C // P      # channel tiles (4)
T = 4096         # tokens per sample (h*w)
NQ = 2048        # query tokens per core
JC = 512         # query chunk (PSUM bank width in fp32)
NJ = NQ // JC    # query chunks per core (4)
IT = T // P      # key tiles (32)
NG_TILE = 8      # groups per channel tile (32 groups / 4 tiles)
GS = 16          # channels per group
EPS = 1e-6
SCALE = 1.0 / np.sqrt(C)
INV_SQRT2 = 0.7071067811865476
GN_MODE = "sums"  # "sums" (reduce+Square) or "bn" (bn_stats)


def _emit_body(nc, pools, aps, nj=NJ, do_attn=True):
    """One full forward pass. `pools` are long-lived tile pools; PSUM usage
    never exceeds 8 banks (pss 2 + psav 4 + psz 2)."""
    (const, big, sbx, st, sbe, sbw, sbq, sby, pss, psav, psz) = pools
    (x_d, xb_d, y_d, w_sb, vec_sb, bv_row, indf, indb, ones_f1,
     ones_fc, eps_t) = aps

    # ---- persistent activations ----
    qf = big.tile([P, CT, NQ], BF16, tag="qf")    # Q  [c, j]
    vfT = big.tile([P, IT, C], BF16, tag="vfT")   # V^T [i, c]
    # GroupNorm is folded into the projections: w2 = w * diag(a) per c_in.
    # K is never projected: S = xn^T (Wk^T Wq) xn, with M0 = Wk^T Wq from
    # the host; m2 = diag(a) applied to M0^T rows here, the output-side
    # diag(a) applied in the Q-copy ACT scale.
    w2 = {n: big.tile([P, CT, C], BF16, tag=f"w2{n}", name=f"w2{n}")
          for n in ("m0T", "wvT")}

    # ================= phase 1: GroupNorm =================
    # stats + normalization read the host-cast bf16 copy of x (half the HBM
    # traffic of f32; stats arithmetic stays f32)
    x_ts = []
    for t in range(CT):
        x_t = sbx.tile([P, T], BF16, tag="x", name=f"x{t}")
        # DMA in halves so stats can start on the first half early
        nc.sync.dma_start(
            out=x_t[:, 0:T // 2], in_=xb_d[t * P:(t + 1) * P, 0:T // 2])
        nc.sync.dma_start(
            out=x_t[:, T // 2:T], in_=xb_d[t * P:(t + 1) * P, T // 2:T])
        x_ts.append(x_t)
    b_bfs = []
    a_ts = []
    gm_all = st.tile([NG_TILE, 2, CT], F32, tag="gm_all")   # [Mg | rstd] x tile
    var_all = st.tile([NG_TILE, CT], F32, tag="var_all")
    for t in range(CT):
        x_t = x_ts[t]
        t2 = st.tile([P, 2], F32, tag="t2", bufs=4)
        if GN_MODE == "sums":
            # raw per-channel sums: sum(x) on DVE, sum(x^2) on ACT (parallel
            # engines, per half so each overlaps the other half's DMA)
            parts = st.tile([P, 4], F32, tag="parts", bufs=4)
            for h in range(2):
                sl = slice(h * (T // 2), (h + 1) * (T // 2))
                nc.vector.reduce_sum(
                    out=parts[:, h:h + 1], in_=x_t[:, sl],
                    axis=mybir.AxisListType.X)
                sq_scr = st.tile([P, T // 2], BF16, tag="sqscr")
                nc.scalar.activation(
                    out=sq_scr, in_=x_t[:, sl], func=AF.Square,
                    accum_out=parts[:, 2 + h:3 + h])
            # t2 = [sum(x), sum(x^2)]; indf carries the 1/(16*4096) factor
            nc.vector.tensor_add(t2[:, 0:1], parts[:, 0:1], parts[:, 1:2])
            nc.vector.tensor_add(t2[:, 1:2], parts[:, 2:3], parts[:, 3:4])
        else:
            stats = st.tile([P, 8, 6], F32, tag="stats")
            for sg in range(8):
                nc.vector.bn_stats(
                    out=stats[:, sg, :], in_=x_t[:, sg * 512:(sg + 1) * 512])
            mv = st.tile([P, 2], F32, tag="mv")
            nc.vector.bn_aggr(out=mv, in_=stats)
            # t2 = [mean, E[x^2]] * (GS*T) to match the indf scaling
            nc.vector.tensor_scalar_mul(
                out=t2[:, 0:1], in0=mv[:, 0:1], scalar1=float(GS * T))
            nc.vector.tensor_mul(t2[:, 1:2], mv[:, 0:1], mv[:, 0:1])
            nc.vector.tensor_add(t2[:, 1:2], t2[:, 1:2], mv[:, 1:2])
            nc.vector.tensor_scalar_mul(
                out=t2[:, 1:2], in0=t2[:, 1:2], scalar1=float(GS * T))
        # group-reduce -> [mean_g, E[x^2]_g]
        g_ps = psz.tile([NG_TILE, 2], F32, tag="z")
        nc.tensor.matmul(g_ps, indf, t2, start=True, stop=True)
        nc.vector.tensor_copy(out=gm_all[:, :, t:t + 1], in_=g_ps.rearrange("p (s o) -> p s o", o=1))
        nc.vector.tensor_mul(
            var_all[:, t:t + 1], gm_all[:, 0, t:t + 1], gm_all[:, 0, t:t + 1])
        nc.vector.tensor_sub(
            var_all[:, t:t + 1], gm_all[:, 1, t:t + 1], var_all[:, t:t + 1])

    # batched tail: ONE Sqrt / reciprocal / broadcast for all 4 tiles
    # (avoids ScalarE LUT reloads between Square/Sqrt/Identity)
    sd_all = st.tile([NG_TILE, CT], F32, tag="sd_all")
    nc.scalar.activation(
        out=sd_all, in_=var_all, func=AF.Sqrt, bias=eps_t[0:NG_TILE, :])
    nc.vector.reciprocal(out=gm_all[:, 1, :], in_=sd_all)
    bc_ps = psz.tile([P, 2, CT], F32, tag="z")
    nc.tensor.matmul(bc_ps, indb, gm_all, start=True, stop=True)
    for t in range(CT):
        a_t = st.tile([P, 1], F32, tag="a", bufs=4)
        b_t = st.tile([P, 1], F32, tag="b", bufs=6)
        nc.vector.tensor_mul(
            a_t, bc_ps[:, 1, t:t + 1], vec_sb["gamma"][:, t:t + 1])
        nc.vector.tensor_mul(b_t, bc_ps[:, 0, t:t + 1], a_t)
        nc.vector.tensor_sub(b_t, vec_sb["beta"][:, t:t + 1], b_t)
        b_bf = st.tile([P, 1], BF16, tag="bbf", bufs=6)
        nc.vector.tensor_copy(out=b_bf, in_=b_t)
        b_bfs.append(b_bf)
        a_ts.append(a_t)
        # fold the GN affine scale into the projection weights (per c_in row)
        for n in ("m0T", "wvT"):
            nc.scalar.activation(
                out=w2[n][:, t, :], in_=w_sb[n][:, t, :],
                func=AF.Identity, scale=a_t)

    # ---- beta_q = bq + Wq @ b_gn  (the only projection bias that survives:
    # per-query score shifts cancel in softmax; the per-key shift r is
    # handled below as an exp() bias)
    bq_bf = st.tile([P, CT], BF16, tag="bq_bf")
    for mo in range(CT):
        bp = psz.tile([P, 1], F32, tag="z", name="bp")
        for t in range(CT):
            nc.tensor.matmul(
                bp, w_sb["wqT"][:, t, mo * P:(mo + 1) * P], b_bfs[t],
                start=(t == 0), stop=(t == CT - 1))
        tmp = st.tile([P, 1], F32, tag="bqtmp")
        nc.vector.tensor_add(tmp, bp, vec_sb["bq"][:, mo:mo + 1])
        nc.vector.tensor_copy(out=bq_bf[:, mo:mo + 1], in_=tmp)
    # v_r = a * (Wk^T beta_q)   (per-key shift direction)
    v_bf = st.tile([P, CT], BF16, tag="v_bf")
    for mt in range(CT):
        w1 = psz.tile([P, 1], F32, tag="z", name="w1")
        for kt in range(CT):
            nc.tensor.matmul(
                w1, w_sb["wkR"][:, kt, mt * P:(mt + 1) * P],
                bq_bf[:, kt:kt + 1],
                start=(kt == 0), stop=(kt == CT - 1))
        tmp2 = st.tile([P, 1], F32, tag="vtmp")
        nc.vector.tensor_mul(tmp2, w1, a_ts[mt])
        nc.vector.tensor_copy(out=v_bf[:, mt:mt + 1], in_=tmp2)
    # r[i] = v_r^T x_i for all keys, assembled as [128, IT] for exp bias
    # (row -> partition-major via a bf16 DMA transpose through DRAM)
    r_dram = nc.dram_tensor("r_scratch", [T], BF16, kind="ExternalOutput").ap()
    r_full = st.tile([1, T], BF16, tag="rfull")
    for ic in range(T // JC):
        r_ps = psz.tile([1, JC], F32, tag="z", name="rps")
        for t in range(CT):
            nc.tensor.matmul(
                r_ps, v_bf[:, t:t + 1], x_ts[t][:, ic * JC:(ic + 1) * JC],
                start=(t == 0), stop=(t == CT - 1))
        nc.vector.tensor_scalar_mul(
            out=r_full[:, ic * JC:(ic + 1) * JC], in0=r_ps, scalar1=SCALE)
    nc.sync.dma_start(
        out=r_dram.rearrange("(o t) -> o t", o=1), in_=r_full)
    r_ptb = st.tile([P, IT], BF16, tag="rptb")
    nc.sync.dma_start_transpose(
        out=r_ptb, in_=r_dram.rearrange("(k p) -> k p", p=P))
    r_pt = st.tile([P, IT], F32, tag="rpt")
    nc.vector.tensor_copy(out=r_pt, in_=r_ptb)
    # V bias as a row, broadcast across partitions via PE
    bvr_ps = psz.tile([1, C], F32, tag="z")
    for t in range(CT):
        nc.tensor.matmul(bvr_ps, b_bfs[t], w_sb["wvT"][:, t, :],
                         start=(t == 0), stop=(t == CT - 1))
    bvrow_sb = st.tile([1, C], F32, tag="bvrow")
    nc.vector.tensor_add(bvrow_sb, bvr_ps, bv_row)
    bvrep_ps = psz.tile([P, C], F32, tag="z")
    nc.tensor.matmul(bvrep_ps, ones_f1, bvrow_sb, start=True, stop=True)
    bvrep = sbw.tile([P, C], F32, tag="bvrep")
    nc.vector.tensor_copy(out=bvrep, in_=bvrep_ps)

    # ================= phase 2: Q/K/V projections =================
    # PSUM: alternate between the two pools -> 6 effective buffers.
    def proj_psum(idx, shape):
        pool, tag = ((pss, "s"), (psav, "av"))[idx % 2]
        return pool.tile(shape, F32, tag=tag, name=f"pp{idx % 2}")

    # Q'' = diag(a) M0 diag(a) x  (keys are raw x; no K projection)
    pidx = 0
    for t_out in range(CT):
        for jc in range(NJ):
            qp = proj_psum(pidx, [P, JC]); pidx += 1
            for t in range(CT):
                nc.tensor.matmul(
                    qp,
                    w2["m0T"][:, t, t_out * P:(t_out + 1) * P],
                    x_ts[t][:, jc * JC:(jc + 1) * JC],
                    start=(t == 0), stop=(t == CT - 1),
                )
            nc.scalar.activation(
                out=qf[:, t_out, jc * JC:(jc + 1) * JC], in_=qp,
                func=AF.Identity, scale=a_ts[t_out],
            )
    # V, produced transposed: vfT[i, c] = sum_c' xn[c', i] wvT[c', c]
    for k in range(IT):
        vp = proj_psum(pidx, [P, C]); pidx += 1
        for t in range(CT):
            nc.tensor.matmul(
                vp,
                x_ts[t][:, k * P:(k + 1) * P],
                w2["wvT"][:, t, :],
                start=(t == 0), stop=(t == CT - 1),
            )
        nc.vector.tensor_add(vfT[:, k, :], vp, bvrep)

    # ================= phase 3: attention =================
    if not do_attn:
        return
    for jc in range(nj):
        # residual input for this chunk (original x, queries 0:NQ)
        xqs = []
        for mo in range(CT):
            xq_t = sbq.tile([P, JC], F32, tag="xq")
            nc.sync.dma_start(
                out=xq_t,
                in_=x_d[mo * P:(mo + 1) * P, jc * JC:(jc + 1) * JC],
            )
            xs = sbq.tile([P, JC], F32, tag="xqs")
            # on DVE so phase-3 ScalarE stays pure-Exp (no LUT reloads)
            nc.vector.tensor_scalar(
                out=xs, in0=xq_t,
                scalar1=INV_SQRT2, scalar2=vec_sb["bo2"][:, mo:mo + 1],
                op0=mybir.AluOpType.mult, op1=mybir.AluOpType.add,
            )
            xqs.append(xs)

        zacc = sbw.tile([P, JC], F32, tag="zacc")
        av_ps = [psav.tile([P, JC], F32, tag="av", name=f"av{m}") for m in range(CT)]

        def av_step(k, e_t):
            if k == 0:
                nc.vector.tensor_copy(out=zacc, in_=e_t)
            else:
                nc.vector.tensor_add(zacc, zacc, e_t)
            for m in range(CT):
                nc.tensor.matmul(
                    av_ps[m],
                    vfT[:, k, m * P:(m + 1) * P],
                    e_t,
                    start=(k == 0), stop=(k == IT - 1),
                )

        prev_e = None
        for k in range(IT):
            s_ps = pss.tile([P, JC], F32, tag="s")
            for t in range(CT):
                nc.tensor.matmul(
                    s_ps,
                    x_ts[t][:, k * P:(k + 1) * P],
                    qf[:, t, jc * JC:(jc + 1) * JC],
                    start=(t == 0), stop=(t == CT - 1),
                )
            e_t = sbe.tile([P, JC], BF16, tag="e")
            nc.scalar.activation(out=e_t, in_=s_ps, func=AF.Exp, scale=SCALE,
                                 bias=r_pt[:, k:k + 1])
            if prev_e is not None:
                av_step(k - 1, prev_e)
            prev_e = e_t
        av_step(IT - 1, prev_e)
        # normalize: out_n = av / Z  (Z = cross-partition sum of zacc via
        # PE; broadcast BEFORE reciprocal so the two matmuls are
        # back-to-back and only one DVE hop remains)
        z_ps = psz.tile([1, JC], F32, tag="z")
        nc.tensor.matmul(z_ps, ones_fc, zacc, start=True, stop=True)
        zr_sb = sbw.tile([1, JC], F32, tag="zinv")
        nc.vector.tensor_copy(out=zr_sb, in_=z_ps)
        zbc_ps = psz.tile([P, JC], F32, tag="z")
        nc.tensor.matmul(zbc_ps, ones_f1, zr_sb, start=True, stop=True)
        zrep = sbw.tile([P, JC], F32, tag="zrep")
        nc.vector.reciprocal(out=zrep, in_=zbc_ps)
        # copy av out UNNORMALIZED (no Z dependency -> overlaps the
        # zsum/zbc/reciprocal chain); 1/Z is applied after the projection,
        # which is linear in j so the order is exact
        out_n = sbw.tile([P, CT, JC], BF16, tag="outn")
        for m in range(CT):
            nc.vector.tensor_copy(out=out_n[:, m, :], in_=av_ps[m])
        for mo in range(CT):
            y_ps = psz.tile([P, JC], F32, tag="z")
            for m in range(CT):
                nc.tensor.matmul(
                    y_ps,
                    w_sb["woT"][:, m, mo * P:(mo + 1) * P],
                    out_n[:, m, :],
                    start=(m == 0), stop=(m == CT - 1),
                )
            t_sb = sby.tile([P, JC], F32, tag="y")
            nc.vector.tensor_mul(t_sb, y_ps, zrep)
            yf = sby.tile([P, JC], F32, tag="y")
            nc.vector.scalar_tensor_tensor(
                out=yf, in0=t_sb, scalar=INV_SQRT2, in1=xqs[mo],
                op0=mybir.AluOpType.mult, op1=mybir.AluOpType.add,
            )
            nc.sync.dma_start(
                out=y_d[mo * P:(mo + 1) * P, jc * JC:(jc + 1) * JC],
                in_=yf,
            )


def build_bass(loop_iters=None, nj=NJ, do_attn=True, gn_mode=None):
    global GN_MODE
    if gn_mode is not None:
        GN_MODE = gn_mode
    """loop_iters=None: single-shot kernel.  loop_iters=R: wrap the body in a
    hardware For_i loop (for on-device timing; everything re-runs each
    iteration, output is idempotent)."""
    nc = bacc.Bacc("TRN2", target_bir_lowering=False, debug=False)

    x_d = nc.dram_tensor("x", [C, T], F32, kind="ExternalInput").ap()
    xb_d = nc.dram_tensor("xb", [C, T], BF16, kind="ExternalInput").ap()
    w_d = {
        n: nc.dram_tensor(n, [C, C], BF16, kind="ExternalInput").ap()
        for n in ("wqT", "wkR", "m0T", "wvT", "woT")
    }
    # per-channel vectors in [p, tile] layout (c = t*128 + p)
    vec_d = {
        n: nc.dram_tensor(n, [P, CT], F32, kind="ExternalInput").ap()
        for n in ("bq", "bk", "bo2", "gamma", "beta")
    }
    bvrow_d = nc.dram_tensor("bv_row", [1, C], F32, kind="ExternalInput").ap()
    indf_d = nc.dram_tensor("indf", [P, NG_TILE], F32, kind="ExternalInput").ap()
    indb_d = nc.dram_tensor("indb", [NG_TILE, P], F32, kind="ExternalInput").ap()
    onesc_d = nc.dram_tensor("ones_fc", [P, 1], F32, kind="ExternalInput").ap()
    onesf_d = nc.dram_tensor("ones_f1", [1, P], F32, kind="ExternalInput").ap()
    y_d = nc.dram_tensor("y", [C, NQ], F32, kind="ExternalOutput").ap()

    with tile.TileContext(nc) as tc:
        with (
            tc.tile_pool(name="const", bufs=1) as const,
            tc.tile_pool(name="big", bufs=1) as big,
            tc.tile_pool(name="sbx", bufs=4) as sbx,
            tc.tile_pool(name="st", bufs=2) as st,
            tc.tile_pool(name="sbe", bufs=4) as sbe,
            tc.tile_pool(name="sbw", bufs=2) as sbw,
            tc.tile_pool(name="sbq", bufs=3) as sbq,
            tc.tile_pool(name="sby", bufs=4) as sby,
            tc.tile_pool(name="pss", bufs=2, space="PSUM") as pss,
            tc.tile_pool(name="psav", bufs=4, space="PSUM") as psav,
            tc.tile_pool(name="psz", bufs=2, space="PSUM") as psz,
        ):
            # ---- constants (loaded once, outside any timing loop; small
            # ones first -- the GN group matmuls need them early, while the
            # big weights aren't read until ~18us in) ----
            indf = const.tile([P, NG_TILE], F32, tag="indf")
            nc.gpsimd.dma_start(out=indf, in_=indf_d)
            indb = const.tile([NG_TILE, P], F32, tag="indb")
            nc.gpsimd.dma_start(out=indb, in_=indb_d)
            vec_sb = {}
            for n, d in vec_d.items():
                vec_sb[n] = const.tile([P, CT], F32, tag=n, name=n)
                nc.gpsimd.dma_start(out=vec_sb[n], in_=d)
            bv_row = const.tile([1, C], F32, tag="bvrow_c")
            nc.gpsimd.dma_start(out=bv_row, in_=bvrow_d)
            ones_f1 = const.tile([1, P], F32, tag="onesf")
            nc.gpsimd.dma_start(out=ones_f1, in_=onesf_d)
            ones_fc = const.tile([P, 1], F32, tag="onesc")
            nc.gpsimd.dma_start(out=ones_fc, in_=onesc_d)
            eps_t = const.tile([P, 1], F32, tag="eps")
            nc.vector.memset(eps_t, EPS)
            w_sb = {}
            for n in ("m0T", "wvT", "wqT", "wkR", "woT"):
                w_sb[n] = const.tile([P, CT, C], BF16, tag=n, name=n)
                nc.gpsimd.dma_start(
                    out=w_sb[n], in_=w_d[n].rearrange("(t p) o -> p t o", p=P))

            pools = (const, big, sbx, st, sbe, sbw, sbq, sby, pss, psav, psz)
            aps = (x_d, xb_d, y_d, w_sb, vec_sb, bv_row, indf, indb,
                   ones_f1, ones_fc, eps_t)
            if loop_iters is None:
                _emit_body(nc, pools, aps, nj=nj, do_attn=do_attn)
            else:
                with tc.For_i(0, loop_iters, 1):
                    _emit_body(nc, pools, aps, nj=nj, do_attn=do_attn)
    nc.compile()
    return nc


def make_in_maps(q, gamma, beta, wq, bq, wk, bk, wv, bv, wo, bo):
    """Host-side prep: per-core permuted x + replicated (pre-transposed) weights."""
    f32 = np.float32
    bf16 = ml_dtypes.bfloat16
    q = np.asarray(q, f32)
    b = q.shape[0]
    x = q.reshape(b, C, T)

    def pt(v):  # [512] -> [128, 4] (c = t*128 + p)
        return np.ascontiguousarray(np.asarray(v, f32).reshape(CT, P).T)

    common = {
        "wqT": np.ascontiguousarray(np.asarray(wq, f32).T).astype(bf16),
        "wkR": np.ascontiguousarray(np.asarray(wk, f32)).astype(bf16),
        "m0T": np.ascontiguousarray(
            (np.asarray(wq, f32).T @ np.asarray(wk, f32))).astype(bf16),
        "wvT": np.ascontiguousarray(np.asarray(wv, f32).T).astype(bf16),
        "woT": np.ascontiguousarray(np.asarray(wo, f32).T).astype(bf16),
        "bq": pt(bq), "bk": pt(bk), "bo2": pt(np.asarray(bo, f32) * INV_SQRT2),
        "gamma": pt(gamma), "beta": pt(beta),
        "bv_row": np.ascontiguousarray(np.asarray(bv, f32).reshape(1, C)),
        "indf": np.ascontiguousarray(
            (np.arange(P)[:, None] // GS == np.arange(NG_TILE)[None, :])
            .astype(f32) / (GS * T)),
        "indb": np.ascontiguousarray(
            (np.arange(P)[None, :] // GS == np.arange(NG_TILE)[:, None])
            .astype(f32)),
        "ones_fc": np.ones((P, 1), f32),
        "ones_f1": np.ones((1, P), f32),
    }
    in_maps = []
    for core in range(8):
        s, half = divmod(core, 2)
        xs = x[s]
        if half == 0:
            xp = xs
        else:
            xp = np.concatenate([xs[:, NQ:], xs[:, :NQ]], axis=1)
        xpc = np.ascontiguousarray(xp)
        in_maps.append({"x": xpc, "xb": xpc.astype(bf16), **common})
    return in_maps


def assemble_output(results, b=4, h=64, w=64):
    out = np.empty((b, C, T), np.float32)
    for core in range(8):
        s, half = divmod(core, 2)
        out[s][:, half * NQ:(half + 1) * NQ] = results[core]["y"]
    return out.reshape(b, C, h, w)


_NC = None


def get_nc():
    global _NC
    if _NC is None:
        _NC = build_bass()
    return _NC


def kernel(**inputs):
    in_maps = make_in_maps(**inputs)
    nc = get_nc()
    try:
        res = run_bass_kernel_spmd(nc, in_maps, core_ids=list(range(8)))
    except Exception:
        # transient NRT device wedges have been observed; one retry usually
        # succeeds after the runtime resets the core
        res = run_bass_kernel_spmd(nc, in_maps, core_ids=list(range(8)))
    return assemble_output(res.results)


if __name__ == "__main__":
    nc = get_nc()
    print("built + compiled ok")


# revision 29
# speedup vs baseline: 2.1028x; 1.2679x over previous
"""Attention2D Trainium2 Bass kernel.

Reference computation (per sample s of 4):
    x  = GroupNorm32(q[s])                      # [512, 4096] (c, hw)
    qp = Wq xn + bq ; kp = Wk xn + bk ; vp = Wv xn + bv
    S[i, j]  = sum_c kp[c, i] qp[c, j] / sqrt(512)
    A[:, j]  = softmax_i(S[:, j])
    out[c,j] = sum_i vp[c, i] A[i, j]
    y        = (Wo out + bo + q[s]) / sqrt(2)

Sharding: 8 cores = 4 samples x 2 query-halves (2048 tokens each).
The host permutes the token axis per core so the core's query half is
always tokens [0:2048) -> every core runs an identical program (SPMD,
no collectives).  Key/value work over all 4096 tokens is duplicated
between the two cores of a sample (cheap relative to attention).

On-chip layout: scores are computed as S[i(keys on partitions), j] so
that exp() is a single ScalarE pass PSUM->SBUF and the softmax
denominator Z[j] = sum_i E[i, j] is a ones-vector matmul on TensorE --
no transposes anywhere.  V is produced directly transposed (vfT[i, c])
by swapping matmul operands.  All matmul operands are bf16 (full PE
speed); accumulation in fp32 PSUM; GroupNorm stats in fp32.
"""

import numpy as np
import ml_dtypes

import concourse.bass as bass
import concourse.bacc as bacc
import concourse.tile as tile
import concourse.mybir as mybir
from concourse.bass_utils import run_bass_kernel_spmd

F32 = mybir.dt.float32
BF16 = mybir.dt.bfloat16
AF = mybir.ActivationFunctionType

P = 128          # partitions
C = 512          # channels
CT = C // P      # channel tiles (4)
T = 4096         # tokens per sample (h*w)
NQ = 2048        # query tokens per core
JC = 512         # query chunk (PSUM bank width in fp32)
NJ = NQ // JC    # query chunks per core (4)
IT = T // P      # key tiles (32)
NG_TILE = 8      # groups per channel tile (32 groups / 4 tiles)
GS = 16          # channels per group
EPS = 1e-6
SCALE = 1.0 / np.sqrt(C)
INV_SQRT2 = 0.7071067811865476
GN_MODE = "sums"  # "sums" (reduce+Square) or "bn" (bn_stats)


def _emit_body(nc, pools, aps, nj=NJ, do_attn=True):
    """One full forward pass. `pools` are long-lived tile pools; PSUM usage
    never exceeds 8 banks (pss 2 + psav 4 + psz 2)."""
    (const, big, sbx, st, sbe, sbw, sbq, sby, pss, psav, psz) = pools
    (x_d, xb_d, y_d, w_sb, vec_sb, bv_row, indf, indb, ones_f1,
     ones_fc, eps_t) = aps

    # ---- persistent activations ----
    qf = big.tile([P, CT, NQ], BF16, tag="qf")    # Q  [c, j]
    vfT = big.tile([P, IT, C], BF16, tag="vfT")   # V^T [i, c]
    # GroupNorm is folded into the projections: w2 = w * diag(a) per c_in.
    # K is never projected: S = xn^T (Wk^T Wq) xn, with M0 = Wk^T Wq from
    # the host; m2 = diag(a) applied to M0^T rows here, the output-side
    # diag(a) applied in the Q-copy ACT scale.
    w2 = {n: big.tile([P, CT, C], BF16, tag=f"w2{n}", name=f"w2{n}")
          for n in ("m0T", "wvT")}

    # ================= phase 1: GroupNorm =================
    # stats + normalization read the host-cast bf16 copy of x (half the HBM
    # traffic of f32; stats arithmetic stays f32)
    x_ts = []
    for t in range(CT):
        x_t = sbx.tile([P, T], BF16, tag="x", name=f"x{t}")
        # DMA in halves so stats can start on the first half early
        nc.sync.dma_start(
            out=x_t[:, 0:T // 2], in_=xb_d[t * P:(t + 1) * P, 0:T // 2])
        nc.sync.dma_start(
            out=x_t[:, T // 2:T], in_=xb_d[t * P:(t + 1) * P, T // 2:T])
        x_ts.append(x_t)
    b_bfs = []
    a_ts = []
    gm_all = st.tile([NG_TILE, 2, CT], F32, tag="gm_all")   # [Mg | rstd] x tile
    var_all = st.tile([NG_TILE, CT], F32, tag="var_all")
    for t in range(CT):
        x_t = x_ts[t]
        t2 = st.tile([P, 2], F32, tag="t2", bufs=4)
        if GN_MODE == "sums":
            # raw per-channel sums: sum(x) on DVE, sum(x^2) on ACT (parallel
            # engines, per half so each overlaps the other half's DMA)
            parts = st.tile([P, 4], F32, tag="parts", bufs=4)
            for h in range(2):
                sl = slice(h * (T // 2), (h + 1) * (T // 2))
                nc.vector.reduce_sum(
                    out=parts[:, h:h + 1], in_=x_t[:, sl],
                    axis=mybir.AxisListType.X)
                sq_scr = st.tile([P, T // 2], BF16, tag="sqscr")
                nc.scalar.activation(
                    out=sq_scr, in_=x_t[:, sl], func=AF.Square,
                    accum_out=parts[:, 2 + h:3 + h])
            # t2 = [sum(x), sum(x^2)]; indf carries the 1/(16*4096) factor
            nc.vector.tensor_add(t2[:, 0:1], parts[:, 0:1], parts[:, 1:2])
            nc.vector.tensor_add(t2[:, 1:2], parts[:, 2:3], parts[:, 3:4])
        else:
            stats = st.tile([P, 8, 6], F32, tag="stats")
            for sg in range(8):
                nc.vector.bn_stats(
                    out=stats[:, sg, :], in_=x_t[:, sg * 512:(sg + 1) * 512])
            mv = st.tile([P, 2], F32, tag="mv")
            nc.vector.bn_aggr(out=mv, in_=stats)
            # t2 = [mean, E[x^2]] * (GS*T) to match the indf scaling
            nc.vector.tensor_scalar_mul(
                out=t2[:, 0:1], in0=mv[:, 0:1], scalar1=float(GS * T))
            nc.vector.tensor_mul(t2[:, 1:2], mv[:, 0:1], mv[:, 0:1])
            nc.vector.tensor_add(t2[:, 1:2], t2[:, 1:2], mv[:, 1:2])
            nc.vector.tensor_scalar_mul(
                out=t2[:, 1:2], in0=t2[:, 1:2], scalar1=float(GS * T))
        # group-reduce -> [mean_g, E[x^2]_g]
        g_ps = psz.tile([NG_TILE, 2], F32, tag="z")
        nc.tensor.matmul(g_ps, indf, t2, start=True, stop=True)
        nc.vector.tensor_copy(out=gm_all[:, :, t:t + 1], in_=g_ps.rearrange("p (s o) -> p s o", o=1))
        nc.vector.tensor_mul(
            var_all[:, t:t + 1], gm_all[:, 0, t:t + 1], gm_all[:, 0, t:t + 1])
        nc.vector.tensor_sub(
            var_all[:, t:t + 1], gm_all[:, 1, t:t + 1], var_all[:, t:t + 1])

    # batched tail: ONE Sqrt / reciprocal / broadcast for all 4 tiles
    # (avoids ScalarE LUT reloads between Square/Sqrt/Identity)
    sd_all = st.tile([NG_TILE, CT], F32, tag="sd_all")
    nc.scalar.activation(
        out=sd_all, in_=var_all, func=AF.Sqrt, bias=eps_t[0:NG_TILE, :])
    nc.vector.reciprocal(out=gm_all[:, 1, :], in_=sd_all)
    bc_ps = psz.tile([P, 2, CT], F32, tag="z")
    nc.tensor.matmul(bc_ps, indb, gm_all, start=True, stop=True)
    for t in range(CT):
        a_t = st.tile([P, 1], F32, tag="a", bufs=4)
        b_t = st.tile([P, 1], F32, tag="b", bufs=6)
        nc.vector.tensor_mul(
            a_t, bc_ps[:, 1, t:t + 1], vec_sb["gamma"][:, t:t + 1])
        nc.vector.tensor_mul(b_t, bc_ps[:, 0, t:t + 1], a_t)
        nc.vector.tensor_sub(b_t, vec_sb["beta"][:, t:t + 1], b_t)
        b_bf = st.tile([P, 1], BF16, tag="bbf", bufs=6)
        nc.vector.tensor_copy(out=b_bf, in_=b_t)
        b_bfs.append(b_bf)
        a_ts.append(a_t)
        # fold the GN affine scale into the projection weights (per c_in row)
        for n in ("m0T", "wvT"):
            nc.scalar.activation(
                out=w2[n][:, t, :], in_=w_sb[n][:, t, :],
                func=AF.Identity, scale=a_t)

    # ---- beta_q = bq + Wq @ b_gn  (the only projection bias that survives:
    # per-query score shifts cancel in softmax; the per-key shift r is
    # handled below as an exp() bias)
    bq_bf = st.tile([P, CT], BF16, tag="bq_bf")
    for mo in range(CT):
        bp = psz.tile([P, 1], F32, tag="z", name="bp")
        for t in range(CT):
            nc.tensor.matmul(
                bp, w_sb["wqT"][:, t, mo * P:(mo + 1) * P], b_bfs[t],
                start=(t == 0), stop=(t == CT - 1))
        tmp = st.tile([P, 1], F32, tag="bqtmp")
        nc.vector.tensor_add(tmp, bp, vec_sb["bq"][:, mo:mo + 1])
        nc.vector.tensor_copy(out=bq_bf[:, mo:mo + 1], in_=tmp)
    # v_r = a * (Wk^T beta_q)   (per-key shift direction)
    v_bf = st.tile([P, CT], BF16, tag="v_bf")
    for mt in range(CT):
        w1 = psz.tile([P, 1], F32, tag="z", name="w1")
        for kt in range(CT):
            nc.tensor.matmul(
                w1, w_sb["wkR"][:, kt, mt * P:(mt + 1) * P],
                bq_bf[:, kt:kt + 1],
                start=(kt == 0), stop=(kt == CT - 1))
        tmp2 = st.tile([P, 1], F32, tag="vtmp")
        nc.vector.tensor_mul(tmp2, w1, a_ts[mt])
        nc.vector.tensor_copy(out=v_bf[:, mt:mt + 1], in_=tmp2)
    # r[i] = v_r^T x_i for all keys, assembled as [128, IT] for exp bias
    # (row -> partition-major via a bf16 DMA transpose through DRAM)
    r_dram = nc.dram_tensor("r_scratch", [T], BF16, kind="ExternalOutput").ap()
    r_full = st.tile([1, T], BF16, tag="rfull")
    for ic in range(T // JC):
        r_ps = psz.tile([1, JC], F32, tag="z", name="rps")
        for t in range(CT):
            nc.tensor.matmul(
                r_ps, v_bf[:, t:t + 1], x_ts[t][:, ic * JC:(ic + 1) * JC],
                start=(t == 0), stop=(t == CT - 1))
        nc.vector.tensor_scalar_mul(
            out=r_full[:, ic * JC:(ic + 1) * JC], in0=r_ps, scalar1=SCALE)
    nc.sync.dma_start(
        out=r_dram.rearrange("(o t) -> o t", o=1), in_=r_full)
    r_ptb = st.tile([P, IT], BF16, tag="rptb")
    nc.sync.dma_start_transpose(
        out=r_ptb, in_=r_dram.rearrange("(k p) -> k p", p=P))
    r_pt = st.tile([P, IT], F32, tag="rpt")
    nc.vector.tensor_copy(out=r_pt, in_=r_ptb)
    # V bias as a row, broadcast across partitions via PE
    bvr_ps = psz.tile([1, C], F32, tag="z")
    for t in range(CT):
        nc.tensor.matmul(bvr_ps, b_bfs[t], w_sb["wvT"][:, t, :],
                         start=(t == 0), stop=(t == CT - 1))
    bvrow_sb = st.tile([1, C], F32, tag="bvrow")
    nc.vector.tensor_add(bvrow_sb, bvr_ps, bv_row)
    bvrep_ps = psz.tile([P, C], F32, tag="z")
    nc.tensor.matmul(bvrep_ps, ones_f1, bvrow_sb, start=True, stop=True)
    bvrep = sbw.tile([P, C], F32, tag="bvrep")
    nc.vector.tensor_copy(out=bvrep, in_=bvrep_ps)

    # ================= phase 2: Q/K/V projections =================
    # PSUM: alternate between the two pools -> 6 effective buffers.
    def proj_psum(idx, shape):
        pool, tag = ((pss, "s"), (psav, "av"))[idx % 2]
        return pool.tile(shape, F32, tag=tag, name=f"pp{idx % 2}")

    # Q'' = diag(a) M0 diag(a) x  (keys are raw x; no K projection)
    pidx = 0
    for t_out in range(CT):
        for jc in range(NJ):
            qp = proj_psum(pidx, [P, JC]); pidx += 1
            for t in range(CT):
                nc.tensor.matmul(
                    qp,
                    w2["m0T"][:, t, t_out * P:(t_out + 1) * P],
                    x_ts[t][:, jc * JC:(jc + 1) * JC],
                    start=(t == 0), stop=(t == CT - 1),
                )
            nc.scalar.activation(
                out=qf[:, t_out, jc * JC:(jc + 1) * JC], in_=qp,
                func=AF.Identity, scale=a_ts[t_out],
            )
    # V, produced transposed: vfT[i, c] = sum_c' xn[c', i] wvT[c', c]
    for k in range(IT):
        vp = proj_psum(pidx, [P, C]); pidx += 1
        for t in range(CT):
            nc.tensor.matmul(
                vp,
                x_ts[t][:, k * P:(k + 1) * P],
                w2["wvT"][:, t, :],
                start=(t == 0), stop=(t == CT - 1),
            )
        nc.vector.tensor_add(vfT[:, k, :], vp, bvrep)

    # ================= phase 3: attention =================
    if not do_attn:
        return
    for jc in range(nj):
        # residual input for this chunk (original x, queries 0:NQ)
        xqs = []
        for mo in range(CT):
            xq_t = sbq.tile([P, JC], F32, tag="xq")
            nc.sync.dma_start(
                out=xq_t,
                in_=x_d[mo * P:(mo + 1) * P, jc * JC:(jc + 1) * JC],
            )
            xs = sbq.tile([P, JC], F32, tag="xqs")
            # on DVE so phase-3 ScalarE stays pure-Exp (no LUT reloads)
            nc.vector.tensor_scalar(
                out=xs, in0=xq_t,
                scalar1=INV_SQRT2, scalar2=vec_sb["bo2"][:, mo:mo + 1],
                op0=mybir.AluOpType.mult, op1=mybir.AluOpType.add,
            )
            xqs.append(xs)

        zacc = sbw.tile([P, JC], F32, tag="zacc")
        av_ps = [psav.tile([P, JC], F32, tag="av", name=f"av{m}") for m in range(CT)]

        def av_step(k, e_t):
            if k == 0:
                nc.vector.tensor_copy(out=zacc, in_=e_t)
            else:
                nc.vector.tensor_add(zacc, zacc, e_t)
            for m in range(CT):
                nc.tensor.matmul(
                    av_ps[m],
                    vfT[:, k, m * P:(m + 1) * P],
                    e_t,
                    start=(k == 0), stop=(k == IT - 1),
                )

        prev_e = None
        for k in range(IT):
            s_ps = pss.tile([P, JC], F32, tag="s")
            for t in range(CT):
                nc.tensor.matmul(
                    s_ps,
                    x_ts[t][:, k * P:(k + 1) * P],
                    qf[:, t, jc * JC:(jc + 1) * JC],
                    start=(t == 0), stop=(t == CT - 1),
                )
            e_t = sbe.tile([P, JC], BF16, tag="e")
            nc.scalar.activation(out=e_t, in_=s_ps, func=AF.Exp, scale=SCALE,
                                 bias=r_pt[:, k:k + 1])
            if prev_e is not None:
                av_step(k - 1, prev_e)
            prev_e = e_t
        av_step(IT - 1, prev_e)
        # normalize: out_n = av / Z  (Z = cross-partition sum of zacc via
        # PE; broadcast BEFORE reciprocal so the two matmuls are
        # back-to-back and only one DVE hop remains)
        z_ps = psz.tile([1, JC], F32, tag="z")
        nc.tensor.matmul(z_ps, ones_fc, zacc, start=True, stop=True)
        zr_sb = sbw.tile([1, JC], F32, tag="zinv")
        nc.vector.tensor_copy(out=zr_sb, in_=z_ps)
        zbc_ps = psz.tile([P, JC], F32, tag="z")
        nc.tensor.matmul(zbc_ps, ones_f1, zr_sb, start=True, stop=True)
        zrep = sbw.tile([P, JC], F32, tag="zrep")
        nc.vector.reciprocal(out=zrep, in_=zbc_ps)
        # copy av out UNNORMALIZED (no Z dependency -> overlaps the
        # zsum/zbc/reciprocal chain); 1/Z is applied after the projection,
        # which is linear in j so the order is exact
        out_n = sbw.tile([P, CT, JC], BF16, tag="outn")
        for m in range(CT):
            nc.vector.tensor_copy(out=out_n[:, m, :], in_=av_ps[m])
        for mo in range(CT):
            y_ps = psz.tile([P, JC], F32, tag="z")
            for m in range(CT):
                nc.tensor.matmul(
                    y_ps,
                    w_sb["woT"][:, m, mo * P:(mo + 1) * P],
                    out_n[:, m, :],
                    start=(m == 0), stop=(m == CT - 1),
                )
            t_sb = sby.tile([P, JC], F32, tag="y")
            nc.vector.tensor_mul(t_sb, y_ps, zrep)
            yf = sby.tile([P, JC], F32, tag="y")
            nc.vector.scalar_tensor_tensor(
                out=yf, in0=t_sb, scalar=INV_SQRT2, in1=xqs[mo],
                op0=mybir.AluOpType.mult, op1=mybir.AluOpType.add,
            )
            nc.sync.dma_start(
                out=y_d[mo * P:(mo + 1) * P, jc * JC:(jc + 1) * JC],
                in_=yf,
            )


def build_bass(loop_iters=None, nj=NJ, do_attn=True, gn_mode=None):
    global GN_MODE
    if gn_mode is not None:
        GN_MODE = gn_mode
    """loop_iters=None: single-shot kernel.  loop_iters=R: wrap the body in a
    hardware For_i loop (for on-device timing; everything re-runs each
    iteration, output is idempotent)."""
    nc = bacc.Bacc("TRN2", target_bir_lowering=False, debug=False)

    x_d = nc.dram_tensor("x", [C, T], F32, kind="ExternalInput").ap()
    xb_d = nc.dram_tensor("xb", [C, T], BF16, kind="ExternalInput").ap()
    w_d = {
        n: nc.dram_tensor(n, [C, C], BF16, kind="ExternalInput").ap()
        for n in ("wqT", "wkR", "m0T", "wvT", "woT")
    }
    # per-channel vectors in [p, tile] layout (c = t*128 + p)
    vec_d = {
        n: nc.dram_tensor(n, [P, CT], F32, kind="ExternalInput").ap()
        for n in ("bq", "bk", "bo2", "gamma", "beta")
    }
    bvrow_d = nc.dram_tensor("bv_row", [1, C], F32, kind="ExternalInput").ap()
    indf_d = nc.dram_tensor("indf", [P, NG_TILE], F32, kind="ExternalInput").ap()
    indb_d = nc.dram_tensor("indb", [NG_TILE, P], F32, kind="ExternalInput").ap()
    onesc_d = nc.dram_tensor("ones_fc", [P, 1], F32, kind="ExternalInput").ap()
    onesf_d = nc.dram_tensor("ones_f1", [1, P], F32, kind="ExternalInput").ap()
    y_d = nc.dram_tensor("y", [C, NQ], F32, kind="ExternalOutput").ap()

    with tile.TileContext(nc) as tc:
        with (
            tc.tile_pool(name="const", bufs=1) as const,
            tc.tile_pool(name="big", bufs=1) as big,
            tc.tile_pool(name="sbx", bufs=4) as sbx,
            tc.tile_pool(name="st", bufs=2) as st,
            tc.tile_pool(name="sbe", bufs=4) as sbe,
            tc.tile_pool(name="sbw", bufs=2) as sbw,
            tc.tile_pool(name="sbq", bufs=5) as sbq,
            tc.tile_pool(name="sby", bufs=4) as sby,
            tc.tile_pool(name="pss", bufs=2, space="PSUM") as pss,
            tc.tile_pool(name="psav", bufs=4, space="PSUM") as psav,
            tc.tile_pool(name="psz", bufs=2, space="PSUM") as psz,
        ):
            # ---- constants (loaded once, outside any timing loop; small
            # ones first -- the GN group matmuls need them early, while the
            # big weights aren't read until ~18us in) ----
            indf = const.tile([P, NG_TILE], F32, tag="indf")
            nc.gpsimd.dma_start(out=indf, in_=indf_d)
            indb = const.tile([NG_TILE, P], F32, tag="indb")
            nc.gpsimd.dma_start(out=indb, in_=indb_d)
            vec_sb = {}
            for n, d in vec_d.items():
                vec_sb[n] = const.tile([P, CT], F32, tag=n, name=n)
                nc.gpsimd.dma_start(out=vec_sb[n], in_=d)
            bv_row = const.tile([1, C], F32, tag="bvrow_c")
            nc.gpsimd.dma_start(out=bv_row, in_=bvrow_d)
            ones_f1 = const.tile([1, P], F32, tag="onesf")
            nc.gpsimd.dma_start(out=ones_f1, in_=onesf_d)
            ones_fc = const.tile([P, 1], F32, tag="onesc")
            nc.gpsimd.dma_start(out=ones_fc, in_=onesc_d)
            eps_t = const.tile([P, 1], F32, tag="eps")
            nc.vector.memset(eps_t, EPS)
            w_sb = {}
            for n in ("m0T", "wvT", "wqT", "wkR", "woT"):
                w_sb[n] = const.tile([P, CT, C], BF16, tag=n, name=n)
                nc.gpsimd.dma_start(
                    out=w_sb[n], in_=w_d[n].rearrange("(t p) o -> p t o", p=P))

            pools = (const, big, sbx, st, sbe, sbw, sbq, sby, pss, psav, psz)
            aps = (x_d, xb_d, y_d, w_sb, vec_sb, bv_row, indf, indb,
                   ones_f1, ones_fc, eps_t)
            if loop_iters is None:
                _emit_body(nc, pools, aps, nj=nj, do_attn=do_attn)
            else:
                with tc.For_i(0, loop_iters, 1):
                    _emit_body(nc, pools, aps, nj=nj, do_attn=do_attn)
    nc.compile()
    return nc


def make_in_maps(q, gamma, beta, wq, bq, wk, bk, wv, bv, wo, bo):
    """Host-side prep: per-core permuted x + replicated (pre-transposed) weights."""
    f32 = np.float32
    bf16 = ml_dtypes.bfloat16
    q = np.asarray(q, f32)
    b = q.shape[0]
    x = q.reshape(b, C, T)

    def pt(v):  # [512] -> [128, 4] (c = t*128 + p)
        return np.ascontiguousarray(np.asarray(v, f32).reshape(CT, P).T)

    common = {
        "wqT": np.ascontiguousarray(np.asarray(wq, f32).T).astype(bf16),
        "wkR": np.ascontiguousarray(np.asarray(wk, f32)).astype(bf16),
        "m0T": np.ascontiguousarray(
            (np.asarray(wq, f32).T @ np.asarray(wk, f32))).astype(bf16),
        "wvT": np.ascontiguousarray(np.asarray(wv, f32).T).astype(bf16),
        "woT": np.ascontiguousarray(np.asarray(wo, f32).T).astype(bf16),
        "bq": pt(bq), "bk": pt(bk), "bo2": pt(np.asarray(bo, f32) * INV_SQRT2),
        "gamma": pt(gamma), "beta": pt(beta),
        "bv_row": np.ascontiguousarray(np.asarray(bv, f32).reshape(1, C)),
        "indf": np.ascontiguousarray(
            (np.arange(P)[:, None] // GS == np.arange(NG_TILE)[None, :])
            .astype(f32) / (GS * T)),
        "indb": np.ascontiguousarray(
            (np.arange(P)[None, :] // GS == np.arange(NG_TILE)[:, None])
            .astype(f32)),
        "ones_fc": np.ones((P, 1), f32),
        "ones_f1": np.ones((1, P), f32),
    }
    in_maps = []
    for core in range(8):
        s, half = divmod(core, 2)
        xs = x[s]
        if half == 0:
            xp = xs
        else:
            xp = np.concatenate([xs[:, NQ:], xs[:, :NQ]], axis=1)
        xpc = np.ascontiguousarray(xp)
        in_maps.append({"x": xpc, "xb": xpc.astype(bf16), **common})
    return in_maps


def assemble_output(results, b=4, h=64, w=64):
    out = np.empty((b, C, T), np.float32)
    for core in range(8):
        s, half = divmod(core, 2)
        out[s][:, half * NQ:(half + 1) * NQ] = results[core]["y"]
    return out.reshape(b, C, h, w)


_NC = None


def get_nc():
    global _NC
    if _NC is None:
        _NC = build_bass()
    return _NC


def kernel(**inputs):
    in_maps = make_in_maps(**inputs)
    nc = get_nc()
    try:
        res = run_bass_kernel_spmd(nc, in_maps, core_ids=list(range(8)))
    except Exception:
        # transient NRT device wedges have been observed; one retry usually
        # succeeds after the runtime resets the core
        res = run_bass_kernel_spmd(nc, in_maps, core_ids=list(range(8)))
    return assemble_output(res.results)


if __name__ == "__main__":
    nc = get_nc()
    print("built + compiled ok")
